# revision 1
# baseline (speedup 1.0000x reference)
"""Trainium2 Bass kernel for CNN-LSTM-CRF (nn_CNN_LSTM_CRF_8916352106580).

Sharding: data-parallel, one document per NeuronCore (8 docs, 8 cores).
Document b owns paragraphs n in [64b, 64b+64); its LSTM/CRF runs with
batch=1 entirely on its core.  Params are replicated.  Host sums the 8
per-document scalars at the end (the only "collective").

Everything is hardcoded to the problem shapes:
  B=8 docs, T=64 paragraphs/doc, L=64 tokens/para, V=50000, E=768,
  K=256 conv filters x widths (3,4,5), H=64 LSTM hidden, 5 CRF classes.
"""

import os
import sys

sys.path.insert(0, "/opt/trn_rl_repo")

import numpy as np

import concourse.bass as bass
import concourse.mybir as mybir
import concourse.tile as tile
from concourse import bacc
from concourse.masks import make_identity

# ----------------------------------------------------------------------
# ACT function-set steering.  The stock table-choice pass flaps between
# LUT sets (each reload ~1.3us) when e.g. Exp and Ln alternate in the CRF
# loop.  We shrink the *advertised* contents of every set except the two
# we want, so the pass can only pick:
#   sigmoid_and_others          {Sigmoid, Tanh, Copy, Identity, Relu}
#   natural_log_exp_and_others  {Exp, Ln, Copy, Identity, Relu}
# Positions/ids are preserved and advertised sets are subsets of the real
# HW tables, so every emitted act_func_set_id still loads a table that
# really contains the needed function.
# ----------------------------------------------------------------------
import concourse.hw_specs as _hw_specs

_orig_get_tables = _hw_specs.get_activation_tables
_KEEP = {"sigmoid_and_others", "natural_log_exp_and_others"}


def _steered_tables(module_arch):
    tabs = _orig_get_tables(module_arch)
    keep_union = set()
    for name in _KEEP:
        keep_union |= tabs[name]
    out = {}
    for name, funcs in tabs.items():
        if name in _KEEP:
            out[name] = set(funcs)
        else:
            out[name] = set(funcs) - keep_union
    return out


_hw_specs.get_activation_tables = _steered_tables
bacc.get_activation_tables = _steered_tables

# ---------------------------------------------------------------- shapes
B, T, L, V, E, K, H = 8, 64, 64, 50000, 768, 256, 64
NTOK = T * L            # 4096 tokens per document
NGT = 32                # gather tiles of 128 tokens
ECH = E // 128          # 6 embedding chunks
NCLS, SOS, EOS, NEG = 5, 0, 4, -10000.0
SIZES = (3, 4, 5)
POS_CH = NTOK // 512    # 8 position chunks of 512
XT_COLS = NTOK + 8      # padded so shifted windows stay in range

F32 = mybir.dt.float32
BF16 = mybir.dt.bfloat16
F32R = mybir.dt.float32r
I32 = mybir.dt.int32

# gate permutation: torch order i,f,g,o -> our order i,f,o,g
GATE_PERM = np.concatenate([np.arange(0, 64), np.arange(64, 128),
                            np.arange(192, 256), np.arange(128, 192)])
GI, GF, GO, GG = 0, 1, 2, 3  # column index per gate in [64, 4] layout
# g-gate pre-activations are scaled by 2 (all-sigmoid LSTM: tanh via sigmoid)
GSCALE = np.ones(256, np.float32)
GSCALE[192:256] = 2.0


def _permute_gates(w_t):
    """w_t: (..., 256) gate-last; apply perm + g-gate 2x prescale."""
    return (w_t[..., GATE_PERM] * GSCALE).astype(np.float32)

# ------------------------------------------------- smalls column layout
_cols = {}
_c = 0
def _alloc_cols(name, n):
    global _c
    _cols[name] = (_c, _c + n)
    _c += n

for _ld in ("0f", "0b", "1f", "1b"):
    _alloc_cols(f"whh{_ld}", 256)
for _d in ("f", "b"):
    _alloc_cols(f"wih1{_d}F", 256)   # rows 0:64  = forward-half of input
    _alloc_cols(f"wih1{_d}B", 256)   # rows 0:64  = backward-half of input
_alloc_cols("lb", 16)                # [64, 16]: col = ldir*4 + gate
_alloc_cols("linF", NCLS)            # lin_w.T rows 0:64
_alloc_cols("linB", NCLS)            # lin_w.T rows 64:128
_alloc_cols("lin_b_rep", NCLS)       # [64, 5]
_alloc_cols("onehot_em", NCLS)       # [64, 5]
_alloc_cols("A_rep", 25)             # [64, 25] transition replicated
_alloc_cols("A_lb_rep4", 100)        # [16, 100] (A[i,j]+lin_b[i]) x4
_alloc_cols("paircnt", 25)           # [1, 25] incl. EOS->last fold
_alloc_cols("alpha0", NCLS)          # [1, 5]
_alloc_cols("cb", 6)                 # [128, 6] conv biases per k-chunk
_alloc_cols("ones64", 1)             # [64, 1]
SMALL_COLS = _c

MODE = os.environ.get("KERNEL_MM_DTYPE", "f32r")  # f32r | bf16 | f32


def _conv_mm_dtype():
    return {"f32r": F32R, "bf16": BF16, "f32": F32}[MODE]


def _emb_np_dtype():
    import ml_dtypes
    return ml_dtypes.bfloat16 if MODE == "bf16" else np.float32


# ======================================================================
# device program
# ======================================================================

def build_nc(debug_outputs=False, iters=1):
    nc = bacc.Bacc("TRN2", target_bir_lowering=False, debug=False,
                   enable_asserts=False, num_devices=8)

    mm_dt = _conv_mm_dtype()
    # conv datapath dtype: the whole chain (emb -> gather -> transpose ->
    # xtp, and conv weights) carries this dtype so no casts are needed.
    emb_dt = xt_dt = mm_dt

    # ------------------------------------------------------ DRAM tensors
    emb = nc.dram_tensor("emb", [V, E], emb_dt, kind="ExternalInput")
    ids = nc.dram_tensor("ids", [128, NGT], I32, kind="ExternalInput")
    wconv = {s: nc.dram_tensor(f"w{s}", [s, E, K], emb_dt, kind="ExternalInput")
             for s in SIZES}
    wih0 = {d: nc.dram_tensor(f"wih0{d}", [E, 4 * H], F32, kind="ExternalInput")
            for d in ("f", "b")}
    smalls = nc.dram_tensor("smalls", [128, SMALL_COLS], F32, kind="ExternalInput")
    chain = nc.dram_tensor("chain", [1, 1], F32, kind="ExternalInput")
    out = nc.dram_tensor("out", [1, 1], F32, kind="ExternalOutput")

    dbg = {}
    if debug_outputs:
        dbg["pooled"] = nc.dram_tensor("dbg_pooled", [128, 6, T], F32,
                                       kind="ExternalOutput")
        dbg["G00f"] = nc.dram_tensor("dbg_G00f", [64, 4 * T], F32,
                                     kind="ExternalOutput")
        dbg["H0"] = nc.dram_tensor("dbg_H0", [64, 2 * T], F32,
                                   kind="ExternalOutput")
        dbg["H1"] = nc.dram_tensor("dbg_H1", [64, 2 * T], F32,
                                   kind="ExternalOutput")
        dbg["feats"] = nc.dram_tensor("dbg_feats", [T, NCLS], F32,
                                      kind="ExternalOutput")
        dbg["alpha"] = nc.dram_tensor("dbg_alpha", [1, NCLS], F32,
                                      kind="ExternalOutput")
        dbg["xt0"] = nc.dram_tensor("dbg_xt0", [128, 128], F32,
                                    kind="ExternalOutput")

    with tile.TileContext(nc) as tc:
        _program(nc, tc, emb, ids, wconv, wih0, smalls, chain, out, dbg,
                 mm_dt, xt_dt, iters)

    nc.compile()
    return nc


def _program(nc, tc, emb, ids, wconv, wih0, smalls, chain, out, dbg,
             mm_dt, xt_dt, iters=1):
    from contextlib import ExitStack
    es = ExitStack()

    sb = es.enter_context(tc.tile_pool(name="sb", bufs=1))
    gpool = es.enter_context(tc.tile_pool(name="gather", bufs=4))
    pspool = es.enter_context(tc.tile_pool(name="pst", bufs=2, space="PSUM"))
    cpool = es.enter_context(tc.tile_pool(name="conv_ps", bufs=4, space="PSUM"))
    lpool = es.enter_context(tc.tile_pool(name="lstm_ps", bufs=2, space="PSUM"))

    # ------------------------------------------------------- small loads
    smalls_sb = sb.tile([128, SMALL_COLS], F32, tag="smalls")
    nc.sync.dma_start(smalls_sb[:], smalls.ap())

    def S(name, rows=64):
        lo, hi = _cols[name]
        return smalls_sb[0:rows, lo:hi]

    ids_sb = sb.tile([128, NGT], I32, tag="ids")
    nc.sync.dma_start(ids_sb[:], ids.ap())

    # identity for PE transposes, shipped as a NEFF-embedded const (on-chip
    # generators like memset/affine_select can't produce f32r-typed outputs)
    if xt_dt == BF16:
        import ml_dtypes
        eye_np = np.eye(128, dtype=ml_dtypes.bfloat16)
    else:
        eye_np = np.eye(128, dtype=np.float32)
    ident_dram = nc.inline_tensor(eye_np, name="ident128")
    ident = sb.tile([128, 128], xt_dt, tag="ident")
    ident_src = ident_dram.ap()
    if xt_dt == F32R:
        ident_src = ident_src.bitcast(F32R)
    nc.sync.dma_start(ident[:], ident_src)

    # conv weights -> [128, s, 6, K] per size.  The loads are staggered into
    # the position loop below so the first gathers aren't queued behind 9MB
    # of weight DMA at kernel start.
    w_sb = {}
    for s in SIZES:
        w_sb[s] = sb.tile([128, s, ECH, K], mm_dt, tag=f"w{s}", name=f"w{s}_sb")

    def load_wconv(s, after=None):
        src = wconv[s].ap().rearrange("j (c p) k -> p j c k", p=128)
        inst = nc.sync.dma_start(w_sb[s][:], src)
        if after is not None:
            from concourse.tile_rust import add_dep_helper
            add_dep_helper(inst.ins, after.ins,
                           reason="stagger weight DMA behind gathers")
        return inst

    # layer-0 input weights -> [128, 6, 256] per dir (needed only at LSTM)
    wih0_sb = {}
    for d in ("f", "b"):
        wih0_sb[d] = sb.tile([128, ECH, 4 * H], F32, tag=f"wih0{d}", name=f"wih0{d}_sb")

    def load_wih0(d, after=None):
        src = wih0[d].ap().rearrange("(c p) g -> p c g", p=128)
        inst = nc.sync.dma_start(wih0_sb[d][:], src)
        if after is not None:
            from concourse.tile_rust import add_dep_helper
            add_dep_helper(inst.ins, after.ins,
                           reason="stagger weight DMA behind gathers")
        return inst

    chain_sb = sb.tile([1, 1], F32, tag="chain")
    nc.sync.dma_start(chain_sb[:], chain.ap())

    xtpool = es.enter_context(tc.tile_pool(name="xtp", bufs=3))

    def one_iter(it, prev_out):
        # -------------------------- gather + transpose + conv, per 512-token chunk
        # X^T is a rolling per-pos-chunk buffer [128, 6, 520] (E-major).
        # Conv windows never cross paragraph boundaries, so the shifted reads
        # into cols 512..515 only feed discarded pooling positions (garbage OK).
        pooled = sb.tile([128, 6, T], F32, tag="pooled")

        for pos in range(POS_CH):
            xtp = xtpool.tile([128, ECH, 520], xt_dt, tag="xtp")
            pad = xtp[:, :, 512:520]
            nc.vector.memset(pad.bitcast(F32) if xt_dt == F32R else pad, 0.0)
            for gl in range(4):
                g = pos * 4 + gl
                xg = gpool.tile([128, E], emb.dtype, tag="xg")
                last_gather = nc.gpsimd.indirect_dma_start(
                    out=xg[:],
                    out_offset=None,
                    in_=emb.ap(),
                    in_offset=bass.IndirectOffsetOnAxis(ap=ids_sb[:, g:g + 1],
                                                        axis=0),
                )
                if prev_out is not None and pos == 0 and gl == 0:
                    from concourse.tile_rust import add_dep_helper
                    add_dep_helper(last_gather.ins, prev_out.ins,
                                   reason="serialize timing iterations")
                for c in range(ECH):
                    pst = pspool.tile([128, 128], xt_dt, tag="pst")
                    nc.tensor.transpose(pst[:], xg[:, c * 128:(c + 1) * 128],
                                        ident[:])
                    eng = nc.vector if (c % 2 == 0) else nc.scalar
                    if eng is nc.vector:
                        eng.tensor_copy(xtp[:, c, gl * 128:(gl + 1) * 128], pst[:])
                    else:
                        eng.copy(xtp[:, c, gl * 128:(gl + 1) * 128], pst[:])

            if dbg and pos == 0 and xt_dt != BF16:
                nc.sync.dma_start(dbg["xt0"].ap(), xtp[:, 0, 0:128].bitcast(F32))

            if pos == 0 and it == 0:
                load_wconv(3, after=last_gather)

            for si, s in enumerate(SIZES):
                if pos == 0 and it == 0:
                    # prefetch the next weight set while this one computes;
                    # explicit deps keep the DMA queue clear for gathers
                    if s == 3:
                        load_wconv(4, after=last_gather)
                    elif s == 4:
                        load_wconv(5, after=last_gather)
                    else:
                        load_wih0("f", after=last_gather)
                        load_wih0("b", after=last_gather)
                for kc in range(2):
                    cps = cpool.tile([128, 512], F32, tag="cps")
                    first = True
                    for j in range(s):
                        for c in range(ECH):
                            lhsT = w_sb[s][:, j, c, kc * 128:(kc + 1) * 128]
                            rhs = xtp[:, c, j:j + 512]
                            nc.tensor.matmul(cps[:], lhsT, rhs,
                                             start=first,
                                             stop=(j == s - 1 and c == ECH - 1))
                            first = False
                    # windowed max over valid conv positions of each paragraph
                    view = cps[:].rearrange("p (n q) -> p n q", q=L)[:, :, 0:L - s + 1]
                    nc.vector.tensor_reduce(
                        pooled[:, 2 * si + kc, pos * 8:(pos + 1) * 8],
                        view, axis=mybir.AxisListType.X, op=mybir.AluOpType.max)

        # bias + relu (relu(max+b) == max(relu(conv+b)) since windows valid)
        for ch in range(6):
            nc.scalar.activation(pooled[:, ch, :], pooled[:, ch, :],
                                 mybir.ActivationFunctionType.Relu,
                                 bias=smalls_sb[:, _cols["cb"][0] + ch:
                                                _cols["cb"][0] + ch + 1])

        if dbg:
            nc.sync.dma_start(dbg["pooled"].ap(), pooled[:])

        # ------------------------------------------------------------- LSTM
        # G tiles: input projections + bias, layout [64, 4t+g]
        def input_proj(ldir, rhs_tiles, lhs_slices, tag):
            """rhs_tiles: list of rhs APs [P,T]; lhs_slices: per rhs, fn(g)->lhsT"""
            Gt = sb.tile([64, 4 * T], F32, tag=tag)
            n_in = len(rhs_tiles)
            for g in range(4):
                ps = lpool.tile([64, T], F32, tag="lps")
                for idx, (rhs_ap, lhs_fn) in enumerate(zip(rhs_tiles, lhs_slices)):
                    nc.tensor.matmul(ps[:], lhs_fn(g), rhs_ap,
                                     start=(idx == 0), stop=(idx == n_in - 1))
                bias = smalls_sb[0:64, _cols["lb"][0] + 4 * _LDIDX[ldir] + g:
                                 _cols["lb"][0] + 4 * _LDIDX[ldir] + g + 1]
                gv = Gt[:].rearrange("p (t g) -> p t g", g=4)[:, :, g]
                nc.scalar.activation(gv, ps[:],
                                     mybir.ActivationFunctionType.Identity,
                                     bias=bias)
            return Gt

        _LDIDX = {"0f": 0, "0b": 1, "1f": 2, "1b": 3}

        G = {}
        for d in ("f", "b"):
            rhs_tiles = [pooled[:, c, :] for c in range(ECH)]
            lhs = [(lambda g, _c=c, _d=d:
                    wih0_sb[_d][:, _c, g * 64:(g + 1) * 64]) for c in range(ECH)]
            G["0" + d] = input_proj("0" + d, rhs_tiles, lhs, tag=f"G0{d}")

        # recurrence: fwd and bwd of one layer emitted interleaved so their
        # dependency chains overlap across engines.  Per step and direction:
        #   psum = ident64 @ G[:,4t:4t+4]  (+)  4x Whh-slice @ h   (PE)
        #   sigmoid(psum[:,0:3]) / tanh(psum[:,3:4]) -> gates      (ACT)
        #   ig = i*g ; c = scan(f*c + ig) ; h = o*tanh(c)          (DVE/ACT)
        # h is read from Ht[:, t] by the next step's matmuls directly.
        H_out = {}  # (layer, dir) -> [64, T] hidden states
        ident64 = sb.tile([64, 64], F32, tag="ident64")
        eye64_dram = nc.inline_tensor(np.eye(64, dtype=np.float32), name=f"ident64c_{it}")
        nc.sync.dma_start(ident64[:], eye64_dram.ap())

        def make_dir_state(ldir):
            st = {}
            st["whh"] = S(f"whh{ldir}")
            st["c"] = sb.tile([64, 2], F32, tag=f"c{ldir}",
                              name=f"c{ldir}_{it}")   # ping-pong cell state
            nc.vector.memset(st["c"][:], 0.0)
            st["Ht"] = sb.tile([64, T], F32, tag=f"H{ldir}", name=f"H{ldir}_{it}")
            st["gates"] = sb.tile([64, 4], F32, tag=f"gates{ldir}",
                                  name=f"gates{ldir}_{it}")
            st["tc"] = sb.tile([64, 1], F32, tag=f"tanc{ldir}", name=f"tanc{ldir}_{it}")
            st["ig"] = sb.tile([64, 1], F32, tag=f"ig{ldir}", name=f"ig{ldir}_{it}")
            return st

        def dir_step(st, Gt, t, prev_t, step_idx):
            ps = lpool.tile([64, 4], F32, tag="lps", name=f"rec_ps_{t}_{it}")
            last = prev_t is None
            nc.tensor.matmul(ps[:], ident64[:], Gt[:, 4 * t:4 * t + 4],
                             start=True, stop=last)
            if not last:
                h_prev = st["Ht"][:, prev_t:prev_t + 1]
                for g in range(4):
                    nc.tensor.matmul(ps[:, g:g + 1],
                                     st["whh"][:, g * 64:(g + 1) * 64],
                                     h_prev, start=False, stop=(g == 3))
            # all-sigmoid gates: host pre-scaled the g-gate weights by 2, so
            # sigma(pre_act) = sigma(2x) and tanh(x) = 2*sigma(2x) - 1.
            gates = st["gates"]
            nc.scalar.activation(gates[:], ps[:],
                                 mybir.ActivationFunctionType.Sigmoid)
            c_prev = st["c"][:, step_idx % 2:step_idx % 2 + 1]
            c_new = st["c"][:, (step_idx + 1) % 2:(step_idx + 1) % 2 + 1]
            # ig = i * (2*sg - 1) = 2*(i*sg) - i
            nc.vector.tensor_mul(st["ig"][:], gates[:, GI:GI + 1],
                                 gates[:, GG:GG + 1])
            nc.vector.scalar_tensor_tensor(st["ig"][:], st["ig"][:], 2.0,
                                           gates[:, GI:GI + 1],
                                           op0=mybir.AluOpType.mult,
                                           op1=mybir.AluOpType.subtract)
            nc.vector.tensor_tensor_scan(c_new, gates[:, GF:GF + 1], st["ig"][:],
                                         initial=c_prev,
                                         op0=mybir.AluOpType.mult,
                                         op1=mybir.AluOpType.add)
            # tanh(c) = 2*sigma(2c) - 1;  h = o*tanh(c) = 2*(o*s2c) - o
            nc.scalar.activation(st["tc"][:], c_new,
                                 mybir.ActivationFunctionType.Sigmoid, scale=2.0)
            nc.vector.tensor_mul(st["tc"][:], gates[:, GO:GO + 1], st["tc"][:])
            nc.vector.scalar_tensor_tensor(st["Ht"][:, t:t + 1], st["tc"][:], 2.0,
                                           gates[:, GO:GO + 1],
                                           op0=mybir.AluOpType.mult,
                                           op1=mybir.AluOpType.subtract)

        def run_layer(lf, lb, Gf, Gb):
            stf = make_dir_state(lf)
            stb = make_dir_state(lb)
            for i in range(T):
                dir_step(stf, Gf, i, i - 1 if i else None, i)
                dir_step(stb, Gb, T - 1 - i, T - i if i else None, i)
            H_out[lf] = stf["Ht"]
            H_out[lb] = stb["Ht"]

        run_layer("0f", "0b", G["0f"], G["0b"])

        if dbg:
            nc.sync.dma_start(dbg["G00f"].ap(), G["0f"][:])
            nc.sync.dma_start(dbg["H0"].ap()[:, 0:T], H_out["0f"][:])
            nc.sync.dma_start(dbg["H0"].ap()[:, T:2 * T], H_out["0b"][:])

        for d in ("f", "b"):
            rhs_tiles = [H_out["0f"][:], H_out["0b"][:]]
            lhs = [(lambda g, _h=half, _d=d:
                    S(f"wih1{_d}{_h}")[:, g * 64:(g + 1) * 64])
                   for half in ("F", "B")]
            G["1" + d] = input_proj("1" + d, rhs_tiles, lhs, tag=f"G1{d}")

        run_layer("1f", "1b", G["1f"], G["1b"])

        if dbg:
            nc.sync.dma_start(dbg["H1"].ap()[:, 0:T], H_out["1f"][:])
            nc.sync.dma_start(dbg["H1"].ap()[:, T:2 * T], H_out["1b"][:])

        # ---------------------------------------------------------- linear
        # feats_tc [T, 5] = H1f.T @ linF + H1b.T @ linB  (+ lin_b)
        fps = lpool.tile([T, NCLS], F32, tag="lps")
        nc.tensor.matmul(fps[:], H_out["1f"][:], S("linF"), start=True, stop=False)
        nc.tensor.matmul(fps[:], H_out["1b"][:], S("linB"), start=False, stop=True)
        feats = sb.tile([T, NCLS], F32, tag="feats")
        nc.vector.tensor_add(feats[:], fps[:], S("lin_b_rep"))

        if dbg:
            nc.sync.dma_start(dbg["feats"].ap(), feats[:])

        # ------------------------------------------------------------- CRF
        # Tree reduction in the log semiring, partition-parallel.  The CRF
        # scan is a chain of T log-matrix-products M_t
        # (M_t[i,j] = A[i,j] + lin_b[i] + rawfeat_t[i]); combine adjacent
        # pairs per level:  C = later (x) earlier,
        #   C[i,k] = LSE_j(later[i,j] + earlier[j,k]).
        # Layout: 4 time-consecutive matrices per partition row across 16
        # partitions; two in-row combine levels, one flatten DMA, then the
        # remaining levels on partition 0.
        def crf_ap(base, extra_off, dims):
            pdim = [list(base.ap[0])]
            return bass.AP(base.tensor, base.offset + extra_off,
                           pdim + [list(d) for d in dims])

        # feats_quad[m, 5q+i] = rawfeats[4m+q, i]   (psum, [16, 20])
        fq_ps = lpool.tile([16, 4 * NCLS], F32, tag="lps", name=f"fq_ps_{it}")
        h1f_q = H_out["1f"][:].rearrange("p (m q) -> p q m", q=4)
        h1b_q = H_out["1b"][:].rearrange("p (m q) -> p q m", q=4)
        for q in range(4):
            nc.tensor.matmul(fq_ps[:, 5 * q:5 * q + 5], h1f_q[:, q, :], S("linF"),
                             start=True, stop=False)
            nc.tensor.matmul(fq_ps[:, 5 * q:5 * q + 5], h1b_q[:, q, :], S("linB"),
                             start=False, stop=True)
        # af_quad[m, 25q + 5i + j] = (A[i,j] + lin_b[i]) + rawfeats[4m+q, i]
        af_quad = sb.tile([16, 100], F32, tag="af_quad")
        fq_b = crf_ap(fq_ps[:], 0, [[5, 4], [1, 5], [0, 5]])
        nc.vector.tensor_add(
            af_quad[:].rearrange("p (q i j) -> p q i j", i=NCLS, j=NCLS),
            S("A_lb_rep4", rows=16).rearrange("p (q i j) -> p q i j",
                                              i=NCLS, j=NCLS),
            fq_b)

        lv, mats, parts, lvl = af_quad, 4, 16, 0
        while mats > 1 or parts > 1:
            if mats == 1:
                flat = sb.tile([1, parts * 25], F32, tag="crf_flat")
                nc.sync.dma_start(flat[:], lv[:])
                lv, mats, parts = flat, parts, 1
            np_pairs = mats // 2
            base = lv[:]
            s_t = sb.tile([parts, np_pairs * 125], F32, tag=f"crf_s{lvl}",
                          name=f"crf_s{lvl}_{it}")
            # ISA limit: 3 free dims per DVE op -> one add per output row i
            for i in range(NCLS):
                out_s = crf_ap(s_t[:], 25 * i,
                               [[125, np_pairs], [5, 5], [1, 5]])
                later = crf_ap(base, 25 + 5 * i,
                               [[50, np_pairs], [0, 5], [1, 5]])
                earlier = crf_ap(base, 0,
                                 [[50, np_pairs], [1, 5], [5, 5]])
                nc.vector.tensor_add(out_s, later, earlier)
            mx_t = sb.tile([parts, np_pairs * 25], F32, tag=f"crf_m{lvl}",
                           name=f"crf_m{lvl}_{it}")
            nc.vector.tensor_reduce(
                mx_t[:], s_t[:].rearrange("o (r j) -> o r j", j=NCLS),
                axis=mybir.AxisListType.X, op=mybir.AluOpType.max)
            mx_b = crf_ap(mx_t[:], 0, [[1, np_pairs * 25], [0, 5]])
            nc.vector.tensor_sub(s_t[:].rearrange("o (r j) -> o r j", j=NCLS),
                                 s_t[:].rearrange("o (r j) -> o r j", j=NCLS),
                                 mx_b)
            nc.scalar.activation(s_t[:], s_t[:],
                                 mybir.ActivationFunctionType.Exp)
            se_t = sb.tile([parts, np_pairs * 25], F32, tag=f"crf_e{lvl}",
                           name=f"crf_e{lvl}_{it}")
            nc.vector.tensor_reduce(
                se_t[:], s_t[:].rearrange("o (r j) -> o r j", j=NCLS),
                axis=mybir.AxisListType.X, op=mybir.AluOpType.add)
            nc.scalar.activation(se_t[:], se_t[:],
                                 mybir.ActivationFunctionType.Ln)
            nxt = sb.tile([parts, np_pairs * 25], F32, tag=f"crf_n{lvl}",
                          name=f"crf_n{lvl}_{it}")
            nc.vector.tensor_add(nxt[:], mx_t[:], se_t[:])
            lv = nxt
            mats = np_pairs
            lvl += 1

        # alpha = M_tot (x) alpha0 :  alpha[i] = LSE_j(M_tot[i,j] + alpha0[j])
        alpha = sb.tile([1, NCLS], F32, tag="alpha")
        mx = sb.tile([1, NCLS], F32, tag="crf_m")
        se = sb.tile([1, NCLS], F32, tag="crf_se")
        av = sb.tile([1, 25], F32, tag="crf_av")
        a0b = crf_ap(S("alpha0", rows=1), 0, [[0, 5], [1, 5]])
        nc.vector.tensor_add(av[:].rearrange("o (i j) -> o i j", j=NCLS),
                             lv[:].rearrange("o (i j) -> o i j", j=NCLS), a0b)
        nc.vector.tensor_reduce(mx[:], av[:].rearrange("o (i j) -> o i j", j=NCLS),
                                axis=mybir.AxisListType.X, op=mybir.AluOpType.max)
        nc.vector.tensor_sub(av[:].rearrange("o (i j) -> o i j", j=NCLS),
                             av[:].rearrange("o (i j) -> o i j", j=NCLS),
                             crf_ap(mx[:], 0, [[1, 5], [0, 5]]))
        nc.scalar.activation(av[:], av[:], mybir.ActivationFunctionType.Exp)
        nc.vector.tensor_reduce(se[:], av[:].rearrange("o (i j) -> o i j", j=NCLS),
                                axis=mybir.AxisListType.X, op=mybir.AluOpType.add)
        nc.scalar.activation(se[:], se[:], mybir.ActivationFunctionType.Ln)
        nc.vector.tensor_add(alpha[:], mx[:], se[:])

        if dbg:
            nc.sync.dma_start(dbg["alpha"].ap(), alpha[:])

        # fwd = LSE(alpha + A[EOS, :])
        a_eos = S("A_rep", rows=1)[:, 5 * EOS:5 * EOS + 5]
        nc.vector.tensor_add(se[:], alpha[:], a_eos)   # reuse se as tmp [1,5]
        nc.vector.tensor_reduce(mx[:, 0:1], se[:], axis=mybir.AxisListType.X,
                                op=mybir.AluOpType.max)
        nm = sb.tile([1, 1], F32, tag="crf_nm")
        nc.scalar.mul(nm[:], mx[:, 0:1], -1.0)
        ex5 = sb.tile([1, NCLS], F32, tag="crf_ex5")
        sm1 = sb.tile([1, 1], F32, tag="crf_sm1")
        nc.scalar.activation(ex5[:], se[:], mybir.ActivationFunctionType.Exp,
                             bias=nm[:], accum_out=sm1[:])
        fwd = sb.tile([1, 1], F32, tag="fwd")
        nc.scalar.activation(fwd[:], sm1[:], mybir.ActivationFunctionType.Ln)
        nc.vector.tensor_add(fwd[:], fwd[:], mx[:, 0:1])

        # ------------------------------------------------------------- gold
        em = sb.tile([T, NCLS], F32, tag="em")
        nc.vector.tensor_mul(em[:], feats[:], S("onehot_em"))
        em_r = sb.tile([T, 1], F32, tag="em_r")
        nc.vector.tensor_reduce(em_r[:], em[:], axis=mybir.AxisListType.X,
                                op=mybir.AluOpType.add)
        gps = lpool.tile([1, 1], F32, tag="lps")
        nc.tensor.matmul(gps[:], em_r[:], S("ones64"), start=True, stop=True)

        tr = sb.tile([1, 25], F32, tag="tr")
        nc.vector.tensor_mul(tr[:], S("A_rep", rows=1), S("paircnt", rows=1))
        tr_s = sb.tile([1, 1], F32, tag="tr_s")
        nc.vector.tensor_reduce(tr_s[:], tr[:], axis=mybir.AxisListType.X,
                                op=mybir.AluOpType.add)

        # out = fwd - em_sum - tr_s + 0*chain
        res = sb.tile([1, 1], F32, tag="res")
        nc.vector.tensor_sub(res[:], fwd[:], gps[:])
        nc.vector.tensor_sub(res[:], res[:], tr_s[:])
        zc = sb.tile([1, 1], F32, tag="zc")
        nc.vector.tensor_scalar_mul(zc[:], chain_sb[:], 0.0)
        nc.vector.tensor_add(res[:], res[:], zc[:])
        return nc.sync.dma_start(out.ap(), res[:])


    prev = None
    for _it in range(iters):
        prev = one_iter(_it, prev)
    es.close()


# ======================================================================
# host side
# ======================================================================

def _prep_core_inputs(core, input_ids, tags, emb_t, wconv_t, lstm, lin_w,
                      lin_b, transition):
    """Build the per-core in_map (numpy only)."""
    ids_doc = np.ascontiguousarray(
        input_ids[core * T:(core + 1) * T].reshape(-1).astype(np.int32))
    ids_T = np.ascontiguousarray(ids_doc.reshape(NGT, 128).T)

    smalls = np.zeros((128, SMALL_COLS), np.float32)

    def put(name, arr, rows=None):
        lo, hi = _cols[name]
        r = arr.shape[0] if rows is None else rows
        smalls[0:r, lo:hi] = arr

    for li, (lname, d) in enumerate((("0", "f"), ("0", "b"),
                                     ("1", "f"), ("1", "b"))):
        Wih, Whh, bih, bhh = lstm[lname + d]
        put(f"whh{lname}{d}", _permute_gates(Whh.T))
        b = _permute_gates((bih + bhh))
        lo, hi = _cols["lb"]
        smalls[0:64, lo + 4 * li:lo + 4 * li + 4] = b.reshape(4, 64).T
        if lname == "1":
            WT = _permute_gates(Wih.T)  # (128, 256)
            put(f"wih1{d}F", WT[0:64])
            put(f"wih1{d}B", WT[64:128])

    linT = lin_w.T.astype(np.float32)  # (128, 5)
    put("linF", linT[0:64])
    put("linB", linT[64:128])
    put("lin_b_rep", np.tile(lin_b.astype(np.float32)[None, :], (T, 1)))

    tg = tags[:, core].astype(np.int64)  # (T,)
    onehot = np.zeros((T, NCLS), np.float32)
    onehot[np.arange(T), tg] = 1.0
    put("onehot_em", onehot)

    A = transition.astype(np.float32)
    put("A_rep", np.tile(A.reshape(1, 25), (T, 1)))
    alb = (A + lin_b.astype(np.float32)[:, None]).reshape(1, 25)
    put("A_lb_rep4", np.tile(alb, (16, 4)), rows=16)

    tg_ext = np.concatenate([[SOS], tg])
    paircnt = np.zeros((NCLS, NCLS), np.float32)
    for t in range(T):
        paircnt[tg_ext[t + 1], tg_ext[t]] += 1.0
    paircnt[EOS, tg_ext[T]] += 1.0  # fold A[EOS, last]
    put("paircnt", paircnt.reshape(1, 25), rows=1)

    a0 = np.full((1, NCLS), NEG, np.float32)
    a0[0, SOS] = 0.0
    put("alpha0", a0, rows=1)

    cb = np.stack([lstm["cb"][s] for s in SIZES])  # (3, 256)
    put("cb", cb.reshape(3, 2, 128).transpose(2, 0, 1).reshape(128, 6),
        rows=128)
    put("ones64", np.ones((64, 1), np.float32))

    m = {
        "emb": emb_t,
        "ids": ids_T,
        "smalls": smalls,
        "chain": np.zeros((1, 1), np.float32),
    }
    for s in SIZES:
        m[f"w{s}"] = wconv_t[s]
    for d in ("f", "b"):
        Wih = lstm["0" + d][0]
        m[f"wih0{d}"] = np.ascontiguousarray(_permute_gates(Wih.T))
    return m


_NC_CACHE = {}


def _get_nc(debug_outputs=False):
    key = (MODE, debug_outputs)
    if key not in _NC_CACHE:
        _NC_CACHE[key] = build_nc(debug_outputs)
    return _NC_CACHE[key]


def make_in_maps(inputs):
    """All 8 per-core input maps from the full problem inputs."""
    input_ids = np.asarray(inputs["input_ids"])
    tags = np.asarray(inputs["tags"])
    emb = np.asarray(inputs["emb"])
    transition = np.asarray(inputs["transition"])
    lin_w = np.asarray(inputs["lin_w"])
    lin_b = np.asarray(inputs["lin_b"])

    emb_t = np.ascontiguousarray(emb.astype(_emb_np_dtype()))
    wconv_t = {}
    for s in SIZES:
        w = np.asarray(inputs[f"cw{s}"])[:, 0]          # (256, s, 768)
        wconv_t[s] = np.ascontiguousarray(
            w.transpose(1, 2, 0).astype(_emb_np_dtype()))  # (s, 768, 256)

    lstm = {"cb": {s: np.asarray(inputs[f"cb{s}"]) for s in SIZES}}
    for lname in ("0", "1"):
        for d in ("f", "b"):
            lstm[lname + d] = tuple(
                np.asarray(inputs[f"{k}{lname}{d}"])
                for k in ("Wih", "Whh", "bih", "bhh"))

    return [
        _prep_core_inputs(c, input_ids, tags, emb_t, wconv_t, lstm,
                          lin_w, lin_b, transition)
        for c in range(B)
    ]


def kernel(**inputs):
    from concourse import bass_utils
    nc = _get_nc()
    in_maps = make_in_maps(inputs)
    res = bass_utils.run_bass_kernel_spmd(nc, in_maps, core_ids=list(range(B)))
    total = np.float32(0.0)
    for c in range(B):
        total += np.float32(res.results[c]["out"][0, 0])
    return np.asarray(total, dtype=np.float32)



# revision 34
# speedup vs baseline: 3.7530x; 3.7530x over previous
"""Trainium2 Bass kernel for CNN-LSTM-CRF (nn_CNN_LSTM_CRF_8916352106580).

Sharding: data-parallel, one document per NeuronCore (8 docs, 8 cores).
Document b owns paragraphs n in [64b, 64b+64); its LSTM/CRF runs with
batch=1 entirely on its core.  Params are replicated.  Host sums the 8
per-document scalars at the end (the only "collective").

v2 design vs the f32r baseline:
  * conv runs in fp8e4m3 with DoubleRow perf mode (2 contraction planes
    per matmul, 0.5 cycles/row): host scales emb x16 and conv weights
    x64; the ReLU activation un-scales by 2^-10.
  * conv chunk production (64 paragraphs = 8 chunks of 8) is interleaved
    with LSTM layer-0 consumption: chunks are produced in the order
    0,7,1,6,2,5,3,4 so the fwd chain (ascending) and bwd chain
    (descending) can both start early; production quanta are emitted
    between recurrence steps so the PE never head-blocks.
  * LSTM cell output uses a direct Tanh activation (same ACT table as
    Sigmoid, so no table reloads) instead of the sigmoid(2x) identity,
    shortening the recurrence critical path by one DVE op.

Everything is hardcoded to the problem shapes:
  B=8 docs, T=64 paragraphs/doc, L=64 tokens/para, V=50000, E=768,
  K=256 conv filters x widths (3,4,5), H=64 LSTM hidden, 5 CRF classes.
"""

import os
import sys

sys.path.insert(0, "/opt/trn_rl_repo")

import numpy as np

import concourse.bass as bass
import concourse.mybir as mybir
import concourse.tile as tile
from concourse import bacc
from concourse.tile_rust import add_dep_helper

# ----------------------------------------------------------------------
# ACT function-set steering (same trick as the baseline): restrict the
# advertised tables so the table-choice pass can only pick
#   sigmoid_and_others          {Sigmoid, Tanh, Copy, Identity, Relu}
#   natural_log_exp_and_others  {Exp, Ln, Copy, Identity, Relu}
# ----------------------------------------------------------------------
import concourse.hw_specs as _hw_specs

_orig_get_tables = _hw_specs.get_activation_tables
_KEEP = {"sigmoid_and_others", "natural_log_exp_and_others"}


def _steered_tables(module_arch):
    tabs = _orig_get_tables(module_arch)
    keep_union = set()
    for name in _KEEP:
        keep_union |= tabs[name]
    out = {}
    for name, funcs in tabs.items():
        if name in _KEEP:
            out[name] = set(funcs)
        else:
            out[name] = set(funcs) - keep_union
    return out


_hw_specs.get_activation_tables = _steered_tables
bacc.get_activation_tables = _steered_tables

# ---------------------------------------------------------------- shapes
B, T, L, V, E, K, H = 8, 64, 64, 50000, 768, 256, 64
NTOK = T * L            # 4096 tokens per document
NGT = 32                # gather tiles of 128 tokens
ECH = E // 128          # 6 embedding chunks
NCLS, SOS, EOS, NEG = 5, 0, 4, -10000.0
SIZES = (3, 4, 5)
POS_CH = NTOK // 512    # 8 position chunks of 512 tokens (8 paragraphs)
PPC = 8                 # paragraphs per chunk

F32 = mybir.dt.float32
FP8 = mybir.dt.float8e4
I32 = mybir.dt.int32

EMB_SCALE = 16.0
WCV_SCALE = 64.0
UNSCALE = 1.0 / (EMB_SCALE * WCV_SCALE)

# production order: alternate ends so fwd (ascending) and bwd
# (descending) chains both get their next chunk one block early.
ORDER = (0, 7, 1, 6, 2, 5, 3, 4)

# gate permutation: torch order i,f,g,o -> our order i,f,o,g
GATE_PERM = np.concatenate([np.arange(0, 64), np.arange(64, 128),
                            np.arange(192, 256), np.arange(128, 192)])
# g-gate pre-activations are scaled by 2 (tanh(x) = 2*sigma(2x) - 1)
GSCALE = np.ones(256, np.float32)
GSCALE[192:256] = 2.0


def _permute_gates(w_t):
    """w_t: (..., 256) gate-last; apply perm + g-gate 2x prescale."""
    return (w_t[..., GATE_PERM] * GSCALE).astype(np.float32)

# ------------------------------------------------- smalls column layout
_cols = {}
_c = 0
def _alloc_cols(name, n):
    global _c
    _cols[name] = (_c, _c + n)
    _c += n

for _ld in ("0f", "0b", "1f", "1b"):
    _alloc_cols(f"whh{_ld}", 256)
for _d in ("f", "b"):
    _alloc_cols(f"wih1{_d}F", 256)   # rows 0:64  = forward-half of input
    _alloc_cols(f"wih1{_d}B", 256)   # rows 0:64  = backward-half of input
_alloc_cols("lb", 16)                # [64, 16]: col = ldidx*4 + gate
_alloc_cols("linF", NCLS)            # lin_w.T rows 0:64
_alloc_cols("linB", NCLS)            # lin_w.T rows 64:128
_alloc_cols("lin_b_rep", NCLS)       # [64, 5]
_alloc_cols("onehot_em", NCLS)       # [64, 5]
_alloc_cols("A_rep", 25)             # [64, 25] transition replicated
_alloc_cols("A_lb_rep4", 100)        # [16, 100] (A[i,j]+lin_b[i]) x4
_alloc_cols("paircnt", 25)           # [1, 25] incl. EOS->last fold
_alloc_cols("alpha0", NCLS)          # [1, 5]
_alloc_cols("cb", 6)                 # [128, 6] conv biases per k-chunk
_alloc_cols("ones64", 1)             # [64, 1]
SMALL_COLS = _c

MODE = os.environ.get("KERNEL_MM_DTYPE", "fp8")


def _emb_np_dtype():
    import ml_dtypes
    return ml_dtypes.float8_e4m3


# ======================================================================
# device program
# ======================================================================

def build_nc(debug_outputs=False, iters=1):
    nc = bacc.Bacc("TRN2", target_bir_lowering=False, debug=False,
                   enable_asserts=False, num_devices=8)

    # ------------------------------------------------------ DRAM tensors
    emb = nc.dram_tensor("emb", [V, E], FP8, kind="ExternalInput")
    ids = nc.dram_tensor("ids", [128, NGT], I32, kind="ExternalInput")
    # weights pre-arranged on host to partition-major contiguous layout so
    # each load is one long descriptor per partition
    wconv = {s: nc.dram_tensor(f"w{s}", [128, s * ECH * K], FP8,
                               kind="ExternalInput")
             for s in SIZES}
    wih0 = {d: nc.dram_tensor(f"wih0{d}", [128, ECH * 4 * H], F32,
                              kind="ExternalInput")
            for d in ("f", "b")}
    smalls = nc.dram_tensor("smalls", [128, SMALL_COLS], F32, kind="ExternalInput")
    chain = nc.dram_tensor("chain", [1, 1], F32, kind="ExternalInput")
    out = nc.dram_tensor("out", [1, 1], F32, kind="ExternalOutput")

    dbg = {}
    if debug_outputs:
        dbg["pooled"] = nc.dram_tensor("dbg_pooled", [128, 6, T], F32,
                                       kind="ExternalOutput")
        dbg["G00f"] = nc.dram_tensor("dbg_G00f", [64, 4 * T], F32,
                                     kind="ExternalOutput")
        dbg["H0"] = nc.dram_tensor("dbg_H0", [64, 2 * T], F32,
                                   kind="ExternalOutput")
        dbg["H1"] = nc.dram_tensor("dbg_H1", [64, 2 * T], F32,
                                   kind="ExternalOutput")
        dbg["feats"] = nc.dram_tensor("dbg_feats", [T, NCLS], F32,
                                      kind="ExternalOutput")
        dbg["alpha"] = nc.dram_tensor("dbg_alpha", [1, NCLS], F32,
                                      kind="ExternalOutput")

    with tile.TileContext(nc) as tc:
        _program(nc, tc, emb, ids, wconv, wih0, smalls, chain, out, dbg, iters)

    nc.compile()
    return nc


def _program(nc, tc, emb, ids, wconv, wih0, smalls, chain, out, dbg, iters=1):
    from contextlib import ExitStack
    es = ExitStack()

    sb = es.enter_context(tc.tile_pool(name="sb", bufs=1))
    gpool = es.enter_context(tc.tile_pool(name="gather", bufs=16))
    xtpool = es.enter_context(tc.tile_pool(name="xtp", bufs=4))
    pspool = es.enter_context(tc.tile_pool(name="pst", bufs=2, space="PSUM"))
    cpool = es.enter_context(tc.tile_pool(name="conv_ps", bufs=2, space="PSUM"))
    lpool = es.enter_context(tc.tile_pool(name="lstm_ps", bufs=2, space="PSUM"))
    gppool = es.enter_context(tc.tile_pool(name="gproj_ps", bufs=2, space="PSUM"))

    # ------------------------------------------------------- small loads
    smalls_sb = sb.tile([128, SMALL_COLS], F32, tag="smalls")
    nc.sync.dma_start(smalls_sb[:], smalls.ap())

    def S(name, rows=64):
        lo, hi = _cols[name]
        return smalls_sb[0:rows, lo:hi]

    ids_sb = sb.tile([128, NGT], I32, tag="ids")
    nc.sync.dma_start(ids_sb[:], ids.ap())

    # identities: fp8 for PE transposes, f32 for the G-inject matmul
    import ml_dtypes
    eye8_dram = nc.inline_tensor(np.eye(128, dtype=ml_dtypes.float8_e4m3),
                                 name="ident128_fp8")
    ident8 = sb.tile([128, 128], FP8, tag="ident8")
    nc.sync.dma_start(ident8[:], eye8_dram.ap())
    eye64_dram = nc.inline_tensor(np.eye(64, dtype=np.float32),
                                  name="ident64_f32")
    ident64 = sb.tile([64, 64], F32, tag="ident64")
    nc.sync.dma_start(ident64[:], eye64_dram.ap())

    # conv weights -> [128, s, 6, K] per size; layer-0 input weights
    w_sb = {}
    for s in SIZES:
        w_sb[s] = sb.tile([128, s, ECH, K], FP8, tag=f"w{s}", name=f"w{s}_sb")
    wih0_sb = {}
    for d in ("f", "b"):
        wih0_sb[d] = sb.tile([128, ECH, 4 * H], F32, tag=f"wih0{d}",
                             name=f"wih0{d}_sb")

    def load_weights(after):
        for s in SIZES:
            src = wconv[s].ap().rearrange("p (j c k) -> p j c k", j=s, c=ECH)
            inst = nc.sync.dma_start(w_sb[s][:], src)
            add_dep_helper(inst.ins, after.ins, reason="stagger weight DMA")
        for d in ("f", "b"):
            src = wih0[d].ap().rearrange("p (c g) -> p c g", c=ECH)
            inst = nc.sync.dma_start(wih0_sb[d][:], src)
            add_dep_helper(inst.ins, after.ins, reason="stagger weight DMA")

    chain_sb = sb.tile([1, 1], F32, tag="chain")
    nc.sync.dma_start(chain_sb[:], chain.ap())

    _LDIDX = {"0f": 0, "0b": 1, "1f": 2, "1b": 3}

    def one_iter(it, prev_out):
        pooled = sb.tile([128, ECH, T], F32, tag="pooled", name=f"pooled_{it}")
        G = {ld: sb.tile([64, 4 * T], F32, tag=f"G{ld}", name=f"G{ld}_{it}")
             for ld in ("0f", "0b", "1f", "1b")}

        # -------------------------------------------------- production
        first_gather = [None]

        xtp_of = {}

        def produce_gt(chunk):
            """Generator tier A: gathers + transposes + copies for a chunk."""
            xtp = xtpool.tile([128, ECH, 520], FP8, tag="xtp")
            xtp_of[chunk] = xtp
            nc.gpsimd.memset(xtp[:, :, 512:520], 0.0)
            copy_eng = [nc.vector, nc.scalar, nc.vector, nc.scalar]
            for gl in range(4):
                g = chunk * 4 + gl
                xg = gpool.tile([128, E], FP8, tag="xg")
                gi = nc.gpsimd.indirect_dma_start(
                    out=xg[:],
                    out_offset=None,
                    in_=emb.ap(),
                    in_offset=bass.IndirectOffsetOnAxis(ap=ids_sb[:, g:g + 1],
                                                        axis=0),
                )
                if first_gather[0] is None:
                    first_gather[0] = gi
                    if prev_out is not None:
                        add_dep_helper(gi.ins, prev_out.ins,
                                       reason="serialize timing iterations")
                yield
                # fp8 PE transpose requires output element step of 2
                pst = pspool.tile([128, ECH * 128, 2], FP8, tag="pst")
                for c in range(ECH):
                    nc.tensor.transpose(pst[:, c * 128:(c + 1) * 128, 0],
                                        xg[:, c * 128:(c + 1) * 128],
                                        ident8[:])
                eng = copy_eng[gl]
                src = pst[:, :, 0].rearrange("p (c q) -> p c q", c=ECH)
                if eng is nc.scalar:
                    eng.copy(xtp[:, :, gl * 128:(gl + 1) * 128], src)
                else:
                    eng.tensor_copy(xtp[:, :, gl * 128:(gl + 1) * 128], src)
                yield

        def produce(chunk):
            """Generator tier B: conv + max-pool + relu + G0 projection."""
            xtp = xtp_of[chunk]
            for si, s in enumerate(SIZES):
                for kc in range(2):
                    cps = cpool.tile([128, 512], F32, tag="cps")
                    nmm = 3 * s
                    mi = 0
                    for cp in range(3):
                        for j in range(s):
                            nc.tensor.matmul(
                                cps[:],
                                w_sb[s][:, j, 2 * cp:2 * cp + 2,
                                        kc * 128:(kc + 1) * 128],
                                xtp[:, 2 * cp:2 * cp + 2, j:j + 512],
                                start=(mi == 0), stop=(mi == nmm - 1),
                                perf_mode=mybir.MatmulPerfMode.DoubleRow)
                            mi += 1
                    view = cps[:].rearrange("p (n q) -> p n q", q=L)[:, :, 0:L - s + 1]
                    nc.vector.tensor_reduce(
                        pooled[:, 2 * si + kc, chunk * PPC:(chunk + 1) * PPC],
                        view, axis=mybir.AxisListType.X, op=mybir.AluOpType.max)
                    yield
            # bias + relu + fp8 un-scale
            for ch in range(ECH):
                nc.scalar.activation(
                    pooled[:, ch, chunk * PPC:(chunk + 1) * PPC],
                    pooled[:, ch, chunk * PPC:(chunk + 1) * PPC],
                    mybir.ActivationFunctionType.Relu,
                    bias=smalls_sb[:, _cols["cb"][0] + ch:_cols["cb"][0] + ch + 1],
                    scale=UNSCALE)
            yield
            # layer-0 input projection for this chunk, both dirs x 4 gates
            for d in ("f", "b"):
                for g in range(4):
                    ps = gppool.tile([64, PPC], F32, tag="gps")
                    for c in range(ECH):
                        nc.tensor.matmul(
                            ps[:], wih0_sb[d][:, c, g * 64:(g + 1) * 64],
                            pooled[:, c, chunk * PPC:(chunk + 1) * PPC],
                            start=(c == 0), stop=(c == ECH - 1))
                    gv = G["0" + d][:].rearrange("q (t r) -> q t r", r=4)[
                        :, chunk * PPC:(chunk + 1) * PPC, g]
                    nc.scalar.activation(
                        gv, ps[:], mybir.ActivationFunctionType.Identity,
                        bias=S("lb")[:, 4 * _LDIDX["0" + d] + g:
                                     4 * _LDIDX["0" + d] + g + 1])
                    if g % 2:
                        yield

        agens = []           # tier A: gather/transpose, drained first
        bgens = []           # tier B: conv/pool/G0proj
        done_chunks = set()

        def start(chunk):
            agens.append((chunk, produce_gt(chunk)))
            bgens.append((chunk, produce(chunk)))

        adone = set()

        def _adv(gens, done_set):
            while gens:
                try:
                    next(gens[0][1])
                    return True
                except StopIteration:
                    done_set.add(gens[0][0])
                    gens.pop(0)
            return False

        _PA = int(os.environ.get('PUMP_A', '3'))
        def pump(n, na=None):
            if na is None:
                na = _PA
            for _ in range(na):
                _adv(agens, adone)
            for _ in range(n):
                # a chunk's conv may only emit after its gather tier has
                # fully emitted (tile deps are emission-ordered)
                while bgens and bgens[0][0] not in adone:
                    if not _adv(agens, adone):
                        break
                if not (bgens and bgens[0][0] in adone):
                    return
                _adv(bgens, done_chunks)

        def drain(chunk):
            """Emit any leftover production for `chunk` (and chunks started
            before it, to keep pool allocation order intact)."""
            while chunk not in done_chunks and (agens or bgens):
                pump(1)

        # ------------------------------------------------- LSTM machinery
        GI, GF, GO, GG = 0, 1, 2, 3

        def make_dir_state(ld):
            st = {}
            st["whh"] = S(f"whh{ld}")
            st["c"] = sb.tile([64, 2], F32, tag=f"c{ld}", name=f"c{ld}_{it}")
            nc.vector.memset(st["c"][:], 0.0)
            st["Ht"] = sb.tile([64, T], F32, tag=f"H{ld}", name=f"H{ld}_{it}")
            st["gates"] = sb.tile([64, 4], F32, tag=f"g{ld}", name=f"g{ld}_{it}")
            st["t1"] = sb.tile([64, 1], F32, tag=f"t1{ld}", name=f"t1{ld}_{it}")
            st["ig"] = sb.tile([64, 1], F32, tag=f"ig{ld}", name=f"ig{ld}_{it}")
            st["tc"] = sb.tile([64, 1], F32, tag=f"tc{ld}", name=f"tc{ld}_{it}")
            return st

        def dir_step(st, Gt, t, prev_t, step_idx):
            ps = lpool.tile([64, 4], F32, tag="lps", name=f"lps_{t}_{it}")
            last = prev_t is None
            nc.tensor.matmul(ps[:], ident64[:], Gt[:, 4 * t:4 * t + 4],
                             start=True, stop=last)
            if not last:
                h_prev = st["Ht"][:, prev_t:prev_t + 1]
                for g in range(4):
                    nc.tensor.matmul(ps[:, g:g + 1],
                                     st["whh"][:, g * 64:(g + 1) * 64],
                                     h_prev, start=False, stop=(g == 3))
            gates = st["gates"]
            nc.scalar.activation(gates[:], ps[:],
                                 mybir.ActivationFunctionType.Sigmoid)
            i_ = gates[:, GI:GI + 1]
            f_ = gates[:, GF:GF + 1]
            o_ = gates[:, GO:GO + 1]
            sg = gates[:, GG:GG + 1]
            c_prev = st["c"][:, step_idx % 2:step_idx % 2 + 1]
            c_new = st["c"][:, (step_idx + 1) % 2:(step_idx + 1) % 2 + 1]
            # ig = i * tanh(g) = 2*(i*sg) - i   (host pre-scaled g by 2)
            nc.vector.tensor_mul(st["t1"][:], i_, sg)
            nc.vector.scalar_tensor_tensor(st["ig"][:], st["t1"][:], 2.0, i_,
                                           op0=mybir.AluOpType.mult,
                                           op1=mybir.AluOpType.subtract)
            nc.vector.tensor_tensor_scan(c_new, f_, st["ig"][:],
                                         initial=c_prev,
                                         op0=mybir.AluOpType.mult,
                                         op1=mybir.AluOpType.add)
            nc.scalar.activation(st["tc"][:], c_new,
                                 mybir.ActivationFunctionType.Tanh)
            nc.vector.tensor_mul(st["Ht"][:, t:t + 1], o_, st["tc"][:])

        # --------------------------------------------- phase 1: conv + L0
        # lead-in: produce chunk 0 only, then start the fwd chain at once;
        # the bwd chain starts 8 steps later (after chunk 7 lands).
        # Production of the remaining chunks is pumped between steps.
        start(ORDER[0])
        _adv(agens, adone)       # issue first gather only
        if it == 0:
            load_weights(first_gather[0])
        drain(ORDER[0])
        for c in ORDER[1:]:
            start(c)

        stf = make_dir_state("0f")
        stb = make_dir_state("0b")

        npre = int(os.environ.get("KERNEL_NPRE", "0"))
        for c in ORDER[1:1 + npre]:
            drain(c)

        BLAG = PPC       # bwd starts one chunk behind fwd
        for slot in range(T + BLAG):
            fi = slot
            if fi < T:
                if fi % PPC == 0:
                    drain(fi // PPC)      # fwd chunk deadline
                dir_step(stf, G["0f"], fi, fi - 1 if fi else None, fi)
                pump(3)
            bi = slot - BLAG
            if 0 <= bi < T:
                if bi % PPC == 0:
                    drain(7 - bi // PPC)  # bwd chunk deadline
                pt = T - 1 - bi
                dir_step(stb, G["0b"], pt, pt + 1 if bi else None, bi)
                pump(3)
        pump(1000)

        H0f, H0b = stf["Ht"], stb["Ht"]
        if dbg:
            nc.sync.dma_start(dbg["pooled"].ap(), pooled[:])
            nc.sync.dma_start(dbg["G00f"].ap(), G["0f"][:])
            nc.sync.dma_start(dbg["H0"].ap()[:, 0:T], H0f[:])
            nc.sync.dma_start(dbg["H0"].ap()[:, T:2 * T], H0b[:])

        # ------------------------------------------------- layer-1 input
        for d in ("f", "b"):
            for g in range(4):
                ps = gppool.tile([64, T], F32, tag="gps",
                                 name=f"g1ps{d}{g}_{it}")
                nc.tensor.matmul(ps[:], S(f"wih1{d}F")[:, g * 64:(g + 1) * 64],
                                 H0f[:], start=True, stop=False)
                nc.tensor.matmul(ps[:], S(f"wih1{d}B")[:, g * 64:(g + 1) * 64],
                                 H0b[:], start=False, stop=True)
                gv = G["1" + d][:].rearrange("q (t r) -> q t r", r=4)[:, :, g]
                nc.scalar.activation(
                    gv, ps[:], mybir.ActivationFunctionType.Identity,
                    bias=S("lb")[:, 4 * _LDIDX["1" + d] + g:
                                 4 * _LDIDX["1" + d] + g + 1])

        st1f = make_dir_state("1f")
        st1b = make_dir_state("1b")
        for i in range(T):
            dir_step(st1f, G["1f"], i, i - 1 if i else None, i)
            pt = T - 1 - i
            dir_step(st1b, G["1b"], pt, pt + 1 if i else None, i)
        H1f, H1b = st1f["Ht"], st1b["Ht"]

        if dbg:
            nc.sync.dma_start(dbg["H1"].ap()[:, 0:T], H1f[:])
            nc.sync.dma_start(dbg["H1"].ap()[:, T:2 * T], H1b[:])

        # ---------------------------------------------------------- linear
        fps = lpool.tile([T, NCLS], F32, tag="lps", name=f"fps_{it}")
        nc.tensor.matmul(fps[:], H1f[:], S("linF"), start=True, stop=False)
        nc.tensor.matmul(fps[:], H1b[:], S("linB"), start=False, stop=True)
        feats = sb.tile([T, NCLS], F32, tag="feats", name=f"feats_{it}")
        nc.vector.tensor_add(feats[:], fps[:], S("lin_b_rep"))

        if dbg:
            nc.sync.dma_start(dbg["feats"].ap(), feats[:])

        # ------------------------------------------------------------- CRF
        # Tree reduction in the log semiring, partition-parallel (same as
        # baseline): 4 time-consecutive matrices per partition row across
        # 16 partitions; two in-row combine levels, one flatten DMA, then
        # the remaining levels on partition 0.
        def crf_ap(base, extra_off, dims):
            pdim = [list(base.ap[0])]
            return bass.AP(base.tensor, base.offset + extra_off,
                           pdim + [list(d) for d in dims])

        fq_ps = gppool.tile([16, 4 * NCLS], F32, tag="gps", name=f"fq_ps_{it}")
        h1f_q = H1f[:].rearrange("p (m q) -> p q m", q=4)
        h1b_q = H1b[:].rearrange("p (m q) -> p q m", q=4)
        for q in range(4):
            nc.tensor.matmul(fq_ps[:, 5 * q:5 * q + 5], h1f_q[:, q, :], S("linF"),
                             start=True, stop=False)
            nc.tensor.matmul(fq_ps[:, 5 * q:5 * q + 5], h1b_q[:, q, :], S("linB"),
                             start=False, stop=True)
        af_quad = sb.tile([16, 100], F32, tag="af_quad", name=f"af_quad_{it}")
        fq_b = crf_ap(fq_ps[:], 0, [[5, 4], [1, 5], [0, 5]])
        nc.vector.tensor_add(
            af_quad[:].rearrange("p (q i j) -> p q i j", i=NCLS, j=NCLS),
            S("A_lb_rep4", rows=16).rearrange("p (q i j) -> p q i j",
                                              i=NCLS, j=NCLS),
            fq_b)

        lv, mats, parts, lvl = af_quad, 4, 16, 0
        while mats > 1 or parts > 1:
            if mats == 1:
                flat = sb.tile([1, parts * 25], F32, tag="crf_flat",
                               name=f"crf_flat_{it}")
                nc.sync.dma_start(flat[:], lv[:])
                lv, mats, parts = flat, parts, 1
            np_pairs = mats // 2
            base = lv[:]
            s_t = sb.tile([parts, np_pairs * 125], F32, tag=f"crf_s{lvl}",
                          name=f"crf_s{lvl}_{it}")
            for i in range(NCLS):
                out_s = crf_ap(s_t[:], 25 * i,
                               [[125, np_pairs], [5, 5], [1, 5]])
                later = crf_ap(base, 25 + 5 * i,
                               [[50, np_pairs], [0, 5], [1, 5]])
                earlier = crf_ap(base, 0,
                                 [[50, np_pairs], [1, 5], [5, 5]])
                nc.vector.tensor_add(out_s, later, earlier)
            mx_t = sb.tile([parts, np_pairs * 25], F32, tag=f"crf_m{lvl}",
                           name=f"crf_m{lvl}_{it}")
            nc.vector.tensor_reduce(
                mx_t[:], s_t[:].rearrange("o (r j) -> o r j", j=NCLS),
                axis=mybir.AxisListType.X, op=mybir.AluOpType.max)
            mx_b = crf_ap(mx_t[:], 0, [[1, np_pairs * 25], [0, 5]])
            nc.vector.tensor_sub(s_t[:].rearrange("o (r j) -> o r j", j=NCLS),
                                 s_t[:].rearrange("o (r j) -> o r j", j=NCLS),
                                 mx_b)
            nc.scalar.activation(s_t[:], s_t[:],
                                 mybir.ActivationFunctionType.Exp)
            se_t = sb.tile([parts, np_pairs * 25], F32, tag=f"crf_e{lvl}",
                           name=f"crf_e{lvl}_{it}")
            nc.vector.tensor_reduce(
                se_t[:], s_t[:].rearrange("o (r j) -> o r j", j=NCLS),
                axis=mybir.AxisListType.X, op=mybir.AluOpType.add)
            nc.scalar.activation(se_t[:], se_t[:],
                                 mybir.ActivationFunctionType.Ln)
            nxt = sb.tile([parts, np_pairs * 25], F32, tag=f"crf_n{lvl}",
                          name=f"crf_n{lvl}_{it}")
            nc.vector.tensor_add(nxt[:], mx_t[:], se_t[:])
            lv = nxt
            mats = np_pairs
            lvl += 1

        alpha = sb.tile([1, NCLS], F32, tag="alpha", name=f"alpha_{it}")
        mx = sb.tile([1, NCLS], F32, tag="crf_m", name=f"crf_m_{it}")
        se = sb.tile([1, NCLS], F32, tag="crf_se", name=f"crf_se_{it}")
        av = sb.tile([1, 25], F32, tag="crf_av", name=f"crf_av_{it}")
        a0b = crf_ap(S("alpha0", rows=1), 0, [[0, 5], [1, 5]])
        nc.vector.tensor_add(av[:].rearrange("o (i j) -> o i j", j=NCLS),
                             lv[:].rearrange("o (i j) -> o i j", j=NCLS), a0b)
        nc.vector.tensor_reduce(mx[:], av[:].rearrange("o (i j) -> o i j", j=NCLS),
                                axis=mybir.AxisListType.X, op=mybir.AluOpType.max)
        nc.vector.tensor_sub(av[:].rearrange("o (i j) -> o i j", j=NCLS),
                             av[:].rearrange("o (i j) -> o i j", j=NCLS),
                             crf_ap(mx[:], 0, [[1, 5], [0, 5]]))
        nc.scalar.activation(av[:], av[:], mybir.ActivationFunctionType.Exp)
        nc.vector.tensor_reduce(se[:], av[:].rearrange("o (i j) -> o i j", j=NCLS),
                                axis=mybir.AxisListType.X, op=mybir.AluOpType.add)
        nc.scalar.activation(se[:], se[:], mybir.ActivationFunctionType.Ln)
        nc.vector.tensor_add(alpha[:], mx[:], se[:])

        if dbg:
            nc.sync.dma_start(dbg["alpha"].ap(), alpha[:])

        # fwd = LSE(alpha + A[EOS, :])
        a_eos = S("A_rep", rows=1)[:, 5 * EOS:5 * EOS + 5]
        nc.vector.tensor_add(se[:], alpha[:], a_eos)
        nc.vector.tensor_reduce(mx[:, 0:1], se[:], axis=mybir.AxisListType.X,
                                op=mybir.AluOpType.max)
        nm = sb.tile([1, 1], F32, tag="crf_nm", name=f"crf_nm_{it}")
        nc.scalar.mul(nm[:], mx[:, 0:1], -1.0)
        ex5 = sb.tile([1, NCLS], F32, tag="crf_ex5", name=f"crf_ex5_{it}")
        sm1 = sb.tile([1, 1], F32, tag="crf_sm1", name=f"crf_sm1_{it}")
        nc.scalar.activation(ex5[:], se[:], mybir.ActivationFunctionType.Exp,
                             bias=nm[:], accum_out=sm1[:])
        fwd = sb.tile([1, 1], F32, tag="fwd", name=f"fwd_{it}")
        nc.scalar.activation(fwd[:], sm1[:], mybir.ActivationFunctionType.Ln)
        nc.vector.tensor_add(fwd[:], fwd[:], mx[:, 0:1])

        # ------------------------------------------------------------- gold
        em = sb.tile([T, NCLS], F32, tag="em", name=f"em_{it}")
        nc.vector.tensor_mul(em[:], feats[:], S("onehot_em"))
        em_r = sb.tile([T, 1], F32, tag="em_r", name=f"em_r_{it}")
        nc.vector.tensor_reduce(em_r[:], em[:], axis=mybir.AxisListType.X,
                                op=mybir.AluOpType.add)
        gps = lpool.tile([1, 1], F32, tag="lps", name=f"gold_ps_{it}")
        nc.tensor.matmul(gps[:], em_r[:], S("ones64"), start=True, stop=True)

        tr = sb.tile([1, 25], F32, tag="tr", name=f"tr_{it}")
        nc.vector.tensor_mul(tr[:], S("A_rep", rows=1), S("paircnt", rows=1))
        tr_s = sb.tile([1, 1], F32, tag="tr_s", name=f"tr_s_{it}")
        nc.vector.tensor_reduce(tr_s[:], tr[:], axis=mybir.AxisListType.X,
                                op=mybir.AluOpType.add)

        # out = fwd - em_sum - tr_s + 0*chain
        res = sb.tile([1, 1], F32, tag="res", name=f"res_{it}")
        nc.vector.tensor_sub(res[:], fwd[:], gps[:])
        nc.vector.tensor_sub(res[:], res[:], tr_s[:])
        zc = sb.tile([1, 1], F32, tag="zc", name=f"zc_{it}")
        nc.vector.tensor_scalar_mul(zc[:], chain_sb[:], 0.0)
        nc.vector.tensor_add(res[:], res[:], zc[:])
        return nc.sync.dma_start(out.ap(), res[:])

    prev = None
    for _it in range(iters):
        prev = one_iter(_it, prev)
    es.close()


# ======================================================================
# host side
# ======================================================================

def _prep_core_inputs(core, input_ids, tags, emb_t, wconv_t, lstm, lin_w,
                      lin_b, transition):
    """Build the per-core in_map (numpy only)."""
    ids_doc = np.ascontiguousarray(
        input_ids[core * T:(core + 1) * T].reshape(-1).astype(np.int32))
    ids_T = np.ascontiguousarray(ids_doc.reshape(NGT, 128).T)

    smalls = np.zeros((128, SMALL_COLS), np.float32)

    def put(name, arr, rows=None):
        lo, hi = _cols[name]
        r = arr.shape[0] if rows is None else rows
        smalls[0:r, lo:hi] = arr

    for li, (lname, d) in enumerate((("0", "f"), ("0", "b"),
                                     ("1", "f"), ("1", "b"))):
        Wih, Whh, bih, bhh = lstm[lname + d]
        put(f"whh{lname}{d}", _permute_gates(Whh.T))
        b = _permute_gates((bih + bhh))           # (256,) order i,f,o,g
        lo, hi = _cols["lb"]
        smalls[0:64, lo + 4 * li:lo + 4 * li + 4] = b.reshape(4, 64).T
        if lname == "1":
            WT = _permute_gates(Wih.T)  # (128, 256)
            put(f"wih1{d}F", WT[0:64])
            put(f"wih1{d}B", WT[64:128])

    linT = lin_w.T.astype(np.float32)  # (128, 5)
    put("linF", linT[0:64])
    put("linB", linT[64:128])
    put("lin_b_rep", np.tile(lin_b.astype(np.float32)[None, :], (T, 1)))

    tg = tags[:, core].astype(np.int64)  # (T,)
    onehot = np.zeros((T, NCLS), np.float32)
    onehot[np.arange(T), tg] = 1.0
    put("onehot_em", onehot)

    A = transition.astype(np.float32)
    put("A_rep", np.tile(A.reshape(1, 25), (T, 1)))
    alb = (A + lin_b.astype(np.float32)[:, None]).reshape(1, 25)
    put("A_lb_rep4", np.tile(alb, (16, 4)), rows=16)

    tg_ext = np.concatenate([[SOS], tg])
    paircnt = np.zeros((NCLS, NCLS), np.float32)
    for t in range(T):
        paircnt[tg_ext[t + 1], tg_ext[t]] += 1.0
    paircnt[EOS, tg_ext[T]] += 1.0  # fold A[EOS, last]
    put("paircnt", paircnt.reshape(1, 25), rows=1)

    a0 = np.full((1, NCLS), NEG, np.float32)
    a0[0, SOS] = 0.0
    put("alpha0", a0, rows=1)

    cb = np.stack([lstm["cb"][s] for s in SIZES])  # (3, 256)
    put("cb", cb.reshape(3, 2, 128).transpose(2, 0, 1).reshape(128, 6),
        rows=128)
    put("ones64", np.ones((64, 1), np.float32))

    m = {
        "emb": emb_t,
        "ids": ids_T,
        "smalls": smalls,
        "chain": np.zeros((1, 1), np.float32),
    }
    for s in SIZES:
        # (s, 768, K) -> [128, s*6*K] partition-major contiguous
        w = wconv_t[s].reshape(s, ECH, 128, K).transpose(2, 0, 1, 3)
        m[f"w{s}"] = np.ascontiguousarray(w.reshape(128, -1))
    for d in ("f", "b"):
        Wih = lstm["0" + d][0]
        wp = _permute_gates(Wih.T)                      # (768, 256)
        wp = wp.reshape(ECH, 128, 4 * H).transpose(1, 0, 2)
        m[f"wih0{d}"] = np.ascontiguousarray(wp.reshape(128, -1))
    return m


_NC_CACHE = {}


def _get_nc(debug_outputs=False):
    key = (MODE, debug_outputs)
    if key not in _NC_CACHE:
        _NC_CACHE[key] = build_nc(debug_outputs)
    return _NC_CACHE[key]


def make_in_maps(inputs):
    """All 8 per-core input maps from the full problem inputs."""
    input_ids = np.asarray(inputs["input_ids"])
    tags = np.asarray(inputs["tags"])
    emb = np.asarray(inputs["emb"])
    transition = np.asarray(inputs["transition"])
    lin_w = np.asarray(inputs["lin_w"])
    lin_b = np.asarray(inputs["lin_b"])

    emb_t = np.ascontiguousarray(
        (emb.astype(np.float32) * EMB_SCALE).astype(_emb_np_dtype()))
    wconv_t = {}
    for s in SIZES:
        w = np.asarray(inputs[f"cw{s}"])[:, 0]          # (256, s, 768)
        wconv_t[s] = np.ascontiguousarray(
            (w.transpose(1, 2, 0).astype(np.float32) * WCV_SCALE)
            .astype(_emb_np_dtype()))                    # (s, 768, 256)

    lstm = {"cb": {s: np.asarray(inputs[f"cb{s}"]) for s in SIZES}}
    for lname in ("0", "1"):
        for d in ("f", "b"):
            lstm[lname + d] = tuple(
                np.asarray(inputs[f"{k}{lname}{d}"])
                for k in ("Wih", "Whh", "bih", "bhh"))

    return [
        _prep_core_inputs(c, input_ids, tags, emb_t, wconv_t, lstm,
                          lin_w, lin_b, transition)
        for c in range(B)
    ]


def kernel(**inputs):
    from concourse import bass_utils
    nc = _get_nc()
    in_maps = make_in_maps(inputs)
    res = bass_utils.run_bass_kernel_spmd(nc, in_maps, core_ids=list(range(B)))
    total = np.float32(0.0)
    for c in range(B):
        total += np.float32(res.results[c]["out"][0, 0])
    return np.asarray(total, dtype=np.float32)


# revision 35
# speedup vs baseline: 15.1590x; 4.0392x over previous
"""Trainium2 Bass kernel for CNN-LSTM-CRF (nn_CNN_LSTM_CRF_8916352106580).

Sharding: data-parallel, one document per NeuronCore (8 docs, 8 cores).
Document b owns paragraphs n in [64b, 64b+64); its LSTM/CRF runs with
batch=1 entirely on its core.  Params are replicated.  Host sums the 8
per-document scalars at the end (the only "collective").

Everything is hardcoded to the problem shapes:
  B=8 docs, T=64 paragraphs/doc, L=64 tokens/para, V=50000, E=768,
  K=256 conv filters x widths (3,4,5), H=64 LSTM hidden, 5 CRF classes.
"""

import os
import sys

sys.path.insert(0, "/opt/trn_rl_repo")

import numpy as np

import concourse.bass as bass
import concourse.mybir as mybir
import concourse.tile as tile
from concourse import bacc
from concourse.masks import make_identity

# ----------------------------------------------------------------------
# ACT function-set steering.  The stock table-choice pass flaps between
# LUT sets (each reload ~1.3us) when e.g. Exp and Ln alternate in the CRF
# loop.  We shrink the *advertised* contents of every set except the two
# we want, so the pass can only pick:
#   sigmoid_and_others          {Sigmoid, Tanh, Copy, Identity, Relu}
#   natural_log_exp_and_others  {Exp, Ln, Copy, Identity, Relu}
# Positions/ids are preserved and advertised sets are subsets of the real
# HW tables, so every emitted act_func_set_id still loads a table that
# really contains the needed function.
# ----------------------------------------------------------------------
import concourse.hw_specs as _hw_specs

_orig_get_tables = _hw_specs.get_activation_tables
_KEEP = {"sigmoid_and_others", "natural_log_exp_and_others"}


def _steered_tables(module_arch):
    tabs = _orig_get_tables(module_arch)
    keep_union = set()
    for name in _KEEP:
        keep_union |= tabs[name]
    out = {}
    for name, funcs in tabs.items():
        if name in _KEEP:
            out[name] = set(funcs)
        else:
            out[name] = set(funcs) - keep_union
    return out


_hw_specs.get_activation_tables = _steered_tables
bacc.get_activation_tables = _steered_tables

# ---------------------------------------------------------------- shapes
B, T, L, V, E, K, H = 8, 64, 64, 50000, 768, 256, 64
NTOK = T * L            # 4096 tokens per document
NGT = 32                # gather tiles of 128 tokens
ECH = E // 128          # 6 embedding chunks
NCLS, SOS, EOS, NEG = 5, 0, 4, -10000.0
SIZES = (3, 4, 5)
POS_CH = NTOK // 512    # 8 position chunks of 512
XT_COLS = NTOK + 8      # padded so shifted windows stay in range

F32 = mybir.dt.float32
BF16 = mybir.dt.bfloat16
F32R = mybir.dt.float32r
I32 = mybir.dt.int32

# gate permutation: torch order i,f,g,o -> our order i,f,o,g
GATE_PERM = np.concatenate([np.arange(0, 64), np.arange(64, 128),
                            np.arange(192, 256), np.arange(128, 192)])
GI, GF, GO, GG = 0, 1, 2, 3  # column index per gate in [64, 4] layout
# g-gate pre-activations are scaled by 2 (all-sigmoid LSTM: tanh via sigmoid)
GSCALE = np.ones(256, np.float32)
GSCALE[192:256] = 2.0


def _permute_gates(w_t):
    """w_t: (..., 256) gate-last; apply perm + g-gate 2x prescale."""
    return (w_t[..., GATE_PERM] * GSCALE).astype(np.float32)

# ------------------------------------------------- smalls column layout
_cols = {}
_c = 0
def _alloc_cols(name, n):
    global _c
    _cols[name] = (_c, _c + n)
    _c += n

for _ld in ("0f", "0b", "1f", "1b"):
    _alloc_cols(f"whh{_ld}", 256)
for _d in ("f", "b"):
    _alloc_cols(f"wih1{_d}F", 256)   # rows 0:64  = forward-half of input
    _alloc_cols(f"wih1{_d}B", 256)   # rows 0:64  = backward-half of input
_alloc_cols("lb", 16)                # [64, 16]: col = ldir*4 + gate
_alloc_cols("linF", NCLS)            # lin_w.T rows 0:64
_alloc_cols("linB", NCLS)            # lin_w.T rows 64:128
_alloc_cols("lin_b_rep", NCLS)       # [64, 5]
_alloc_cols("onehot_em", NCLS)       # [64, 5]
_alloc_cols("A_rep", 25)             # [64, 25] transition replicated
_alloc_cols("A_lb_rep4", 100)        # [16, 100] (A[i,j]+lin_b[i]) x4
_alloc_cols("paircnt", 25)           # [1, 25] incl. EOS->last fold
_alloc_cols("alpha0", NCLS)          # [1, 5]
_alloc_cols("cb", 6)                 # [128, 6] conv biases per k-chunk
_alloc_cols("ones64", 1)             # [64, 1]
SMALL_COLS = _c

MODE = os.environ.get("KERNEL_MM_DTYPE", "f32r")  # f32r | bf16 | f32


def _conv_mm_dtype():
    return {"f32r": F32R, "bf16": BF16, "f32": F32}[MODE]


def _emb_np_dtype():
    import ml_dtypes
    return ml_dtypes.bfloat16 if MODE == "bf16" else np.float32


# ======================================================================
# device program
# ======================================================================

def build_nc(debug_outputs=False, iters=1):
    nc = bacc.Bacc("TRN2", target_bir_lowering=False, debug=False,
                   enable_asserts=False, num_devices=8)

    mm_dt = _conv_mm_dtype()
    # conv datapath dtype: the whole chain (emb -> gather -> transpose ->
    # xtp, and conv weights) carries this dtype so no casts are needed.
    emb_dt = xt_dt = mm_dt

    # ------------------------------------------------------ DRAM tensors
    emb = nc.dram_tensor("emb", [V, E], emb_dt, kind="ExternalInput")
    ids = nc.dram_tensor("ids", [128, NGT], I32, kind="ExternalInput")
    wconv = {s: nc.dram_tensor(f"w{s}", [s, E, K], emb_dt, kind="ExternalInput")
             for s in SIZES}
    wih0 = {d: nc.dram_tensor(f"wih0{d}", [E, 4 * H], F32, kind="ExternalInput")
            for d in ("f", "b")}
    smalls = nc.dram_tensor("smalls", [128, SMALL_COLS], F32, kind="ExternalInput")
    chain = nc.dram_tensor("chain", [1, 1], F32, kind="ExternalInput")
    out = nc.dram_tensor("out", [1, 1], F32, kind="ExternalOutput")

    dbg = {}
    if debug_outputs:
        dbg["pooled"] = nc.dram_tensor("dbg_pooled", [128, 6, T], F32,
                                       kind="ExternalOutput")
        dbg["G00f"] = nc.dram_tensor("dbg_G00f", [64, 4 * T], F32,
                                     kind="ExternalOutput")
        dbg["H0"] = nc.dram_tensor("dbg_H0", [64, 2 * T], F32,
                                   kind="ExternalOutput")
        dbg["H1"] = nc.dram_tensor("dbg_H1", [64, 2 * T], F32,
                                   kind="ExternalOutput")
        dbg["feats"] = nc.dram_tensor("dbg_feats", [T, NCLS], F32,
                                      kind="ExternalOutput")
        dbg["alpha"] = nc.dram_tensor("dbg_alpha", [1, NCLS], F32,
                                      kind="ExternalOutput")
        dbg["xt0"] = nc.dram_tensor("dbg_xt0", [128, 128], F32,
                                    kind="ExternalOutput")

    with tile.TileContext(nc) as tc:
        _program(nc, tc, emb, ids, wconv, wih0, smalls, chain, out, dbg,
                 mm_dt, xt_dt, iters)

    nc.compile()
    return nc


def _program(nc, tc, emb, ids, wconv, wih0, smalls, chain, out, dbg,
             mm_dt, xt_dt, iters=1):
    from contextlib import ExitStack
    es = ExitStack()

    sb = es.enter_context(tc.tile_pool(name="sb", bufs=1))
    gpool = es.enter_context(tc.tile_pool(name="gather", bufs=4))
    pspool = es.enter_context(tc.tile_pool(name="pst", bufs=2, space="PSUM"))
    cpool = es.enter_context(tc.tile_pool(name="conv_ps", bufs=4, space="PSUM"))
    lpool = es.enter_context(tc.tile_pool(name="lstm_ps", bufs=2, space="PSUM"))

    # ------------------------------------------------------- small loads
    smalls_sb = sb.tile([128, SMALL_COLS], F32, tag="smalls")
    nc.sync.dma_start(smalls_sb[:], smalls.ap())

    def S(name, rows=64):
        lo, hi = _cols[name]
        return smalls_sb[0:rows, lo:hi]

    ids_sb = sb.tile([128, NGT], I32, tag="ids")
    nc.sync.dma_start(ids_sb[:], ids.ap())

    # identity for PE transposes, shipped as a NEFF-embedded const (on-chip
    # generators like memset/affine_select can't produce f32r-typed outputs)
    if xt_dt == BF16:
        import ml_dtypes
        eye_np = np.eye(128, dtype=ml_dtypes.bfloat16)
    else:
        eye_np = np.eye(128, dtype=np.float32)
    ident_dram = nc.inline_tensor(eye_np, name="ident128")
    ident = sb.tile([128, 128], xt_dt, tag="ident")
    ident_src = ident_dram.ap()
    if xt_dt == F32R:
        ident_src = ident_src.bitcast(F32R)
    nc.sync.dma_start(ident[:], ident_src)

    # conv weights -> [128, s, 6, K] per size.  The loads are staggered into
    # the position loop below so the first gathers aren't queued behind 9MB
    # of weight DMA at kernel start.
    w_sb = {}
    for s in SIZES:
        w_sb[s] = sb.tile([128, s, ECH, K], mm_dt, tag=f"w{s}", name=f"w{s}_sb")

    def load_wconv(s, after=None):
        src = wconv[s].ap().rearrange("j (c p) k -> p j c k", p=128)
        inst = nc.sync.dma_start(w_sb[s][:], src)
        if after is not None:
            from concourse.tile_rust import add_dep_helper
            add_dep_helper(inst.ins, after.ins,
                           reason="stagger weight DMA behind gathers")
        return inst

    # layer-0 input weights -> [128, 6, 256] per dir (needed only at LSTM)
    wih0_sb = {}
    for d in ("f", "b"):
        wih0_sb[d] = sb.tile([128, ECH, 4 * H], F32, tag=f"wih0{d}", name=f"wih0{d}_sb")

    def load_wih0(d, after=None):
        src = wih0[d].ap().rearrange("(c p) g -> p c g", p=128)
        inst = nc.sync.dma_start(wih0_sb[d][:], src)
        if after is not None:
            from concourse.tile_rust import add_dep_helper
            add_dep_helper(inst.ins, after.ins,
                           reason="stagger weight DMA behind gathers")
        return inst

    chain_sb = sb.tile([1, 1], F32, tag="chain")
    nc.sync.dma_start(chain_sb[:], chain.ap())

    xtpool = es.enter_context(tc.tile_pool(name="xtp", bufs=3))

    def one_iter(it, prev_out):
        # -------------------------- gather + transpose + conv, per 512-token chunk
        # X^T is a rolling per-pos-chunk buffer [128, 6, 520] (E-major).
        # Conv windows never cross paragraph boundaries, so the shifted reads
        # into cols 512..515 only feed discarded pooling positions (garbage OK).
        pooled = sb.tile([128, 6, T], F32, tag="pooled")

        for pos in range(POS_CH):
            xtp = xtpool.tile([128, ECH, 520], xt_dt, tag="xtp")
            pad = xtp[:, :, 512:520]
            nc.vector.memset(pad.bitcast(F32) if xt_dt == F32R else pad, 0.0)
            for gl in range(4):
                g = pos * 4 + gl
                xg = gpool.tile([128, E], emb.dtype, tag="xg")
                last_gather = nc.gpsimd.indirect_dma_start(
                    out=xg[:],
                    out_offset=None,
                    in_=emb.ap(),
                    in_offset=bass.IndirectOffsetOnAxis(ap=ids_sb[:, g:g + 1],
                                                        axis=0),
                )
                if prev_out is not None and pos == 0 and gl == 0:
                    from concourse.tile_rust import add_dep_helper
                    add_dep_helper(last_gather.ins, prev_out.ins,
                                   reason="serialize timing iterations")
                for c in range(ECH):
                    pst = pspool.tile([128, 128], xt_dt, tag="pst")
                    nc.tensor.transpose(pst[:], xg[:, c * 128:(c + 1) * 128],
                                        ident[:])
                    eng = nc.vector if (c % 2 == 0) else nc.scalar
                    if eng is nc.vector:
                        eng.tensor_copy(xtp[:, c, gl * 128:(gl + 1) * 128], pst[:])
                    else:
                        eng.copy(xtp[:, c, gl * 128:(gl + 1) * 128], pst[:])

            if dbg and pos == 0 and xt_dt != BF16:
                nc.sync.dma_start(dbg["xt0"].ap(), xtp[:, 0, 0:128].bitcast(F32))

            if pos == 0 and it == 0:
                load_wconv(3, after=last_gather)

            for si, s in enumerate(SIZES):
                if pos == 0 and it == 0:
                    # prefetch the next weight set while this one computes;
                    # explicit deps keep the DMA queue clear for gathers
                    if s == 3:
                        load_wconv(4, after=last_gather)
                    elif s == 4:
                        load_wconv(5, after=last_gather)
                    else:
                        load_wih0("f", after=last_gather)
                        load_wih0("b", after=last_gather)
                for kc in range(2):
                    cps = cpool.tile([128, 512], F32, tag="cps")
                    first = True
                    for j in range(s):
                        for c in range(ECH):
                            lhsT = w_sb[s][:, j, c, kc * 128:(kc + 1) * 128]
                            rhs = xtp[:, c, j:j + 512]
                            nc.tensor.matmul(cps[:], lhsT, rhs,
                                             start=first,
                                             stop=(j == s - 1 and c == ECH - 1))
                            first = False
                    # windowed max over valid conv positions of each paragraph
                    view = cps[:].rearrange("p (n q) -> p n q", q=L)[:, :, 0:L - s + 1]
                    nc.vector.tensor_reduce(
                        pooled[:, 2 * si + kc, pos * 8:(pos + 1) * 8],
                        view, axis=mybir.AxisListType.X, op=mybir.AluOpType.max)

        # bias + relu (relu(max+b) == max(relu(conv+b)) since windows valid)
        for ch in range(6):
            nc.scalar.activation(pooled[:, ch, :], pooled[:, ch, :],
                                 mybir.ActivationFunctionType.Relu,
                                 bias=smalls_sb[:, _cols["cb"][0] + ch:
                                                _cols["cb"][0] + ch + 1])

        if dbg:
            nc.sync.dma_start(dbg["pooled"].ap(), pooled[:])

        # ------------------------------------------------------------- LSTM
        # G tiles: input projections + bias, layout [64, 4t+g]
        def input_proj(ldir, rhs_tiles, lhs_slices, tag):
            """rhs_tiles: list of rhs APs [P,T]; lhs_slices: per rhs, fn(g)->lhsT"""
            Gt = sb.tile([64, 4 * T], F32, tag=tag)
            n_in = len(rhs_tiles)
            for g in range(4):
                ps = lpool.tile([64, T], F32, tag="lps")
                for idx, (rhs_ap, lhs_fn) in enumerate(zip(rhs_tiles, lhs_slices)):
                    nc.tensor.matmul(ps[:], lhs_fn(g), rhs_ap,
                                     start=(idx == 0), stop=(idx == n_in - 1))
                bias = smalls_sb[0:64, _cols["lb"][0] + 4 * _LDIDX[ldir] + g:
                                 _cols["lb"][0] + 4 * _LDIDX[ldir] + g + 1]
                gv = Gt[:].rearrange("p (t g) -> p t g", g=4)[:, :, g]
                nc.scalar.activation(gv, ps[:],
                                     mybir.ActivationFunctionType.Identity,
                                     bias=bias)
            return Gt

        _LDIDX = {"0f": 0, "0b": 1, "1f": 2, "1b": 3}

        G = {}
        for d in ("f", "b"):
            rhs_tiles = [pooled[:, c, :] for c in range(ECH)]
            lhs = [(lambda g, _c=c, _d=d:
                    wih0_sb[_d][:, _c, g * 64:(g + 1) * 64]) for c in range(ECH)]
            G["0" + d] = input_proj("0" + d, rhs_tiles, lhs, tag=f"G0{d}")

        # recurrence: fwd and bwd of one layer emitted interleaved so their
        # dependency chains overlap across engines.  Per step and direction:
        #   psum = ident64 @ G[:,4t:4t+4]  (+)  4x Whh-slice @ h   (PE)
        #   sigmoid(psum[:,0:3]) / tanh(psum[:,3:4]) -> gates      (ACT)
        #   ig = i*g ; c = scan(f*c + ig) ; h = o*tanh(c)          (DVE/ACT)
        # h is read from Ht[:, t] by the next step's matmuls directly.
        H_out = {}  # (layer, dir) -> [64, T] hidden states
        ident64 = sb.tile([64, 64], F32, tag="ident64")
        eye64_dram = nc.inline_tensor(np.eye(64, dtype=np.float32), name=f"ident64c_{it}")
        nc.sync.dma_start(ident64[:], eye64_dram.ap())

        def make_dir_state(ldir):
            st = {}
            st["whh"] = S(f"whh{ldir}")
            st["c"] = sb.tile([64, 2], F32, tag=f"c{ldir}",
                              name=f"c{ldir}_{it}")   # ping-pong cell state
            nc.vector.memset(st["c"][:], 0.0)
            st["Ht"] = sb.tile([64, T], F32, tag=f"H{ldir}", name=f"H{ldir}_{it}")
            st["gates"] = sb.tile([64, 4], F32, tag=f"gates{ldir}",
                                  name=f"gates{ldir}_{it}")
            st["tc"] = sb.tile([64, 1], F32, tag=f"tanc{ldir}", name=f"tanc{ldir}_{it}")
            st["ig"] = sb.tile([64, 1], F32, tag=f"ig{ldir}", name=f"ig{ldir}_{it}")
            return st

        def dir_step(st, Gt, t, prev_t, step_idx):
            ps = lpool.tile([64, 4], F32, tag="lps", name=f"rec_ps_{t}_{it}")
            last = prev_t is None
            nc.tensor.matmul(ps[:], ident64[:], Gt[:, 4 * t:4 * t + 4],
                             start=True, stop=last)
            if not last:
                h_prev = st["Ht"][:, prev_t:prev_t + 1]
                for g in range(4):
                    nc.tensor.matmul(ps[:, g:g + 1],
                                     st["whh"][:, g * 64:(g + 1) * 64],
                                     h_prev, start=False, stop=(g == 3))
            # all-sigmoid gates: host pre-scaled the g-gate weights by 2, so
            # sigma(pre_act) = sigma(2x) and tanh(x) = 2*sigma(2x) - 1.
            gates = st["gates"]
            nc.scalar.activation(gates[:], ps[:],
                                 mybir.ActivationFunctionType.Sigmoid)
            c_prev = st["c"][:, step_idx % 2:step_idx % 2 + 1]
            c_new = st["c"][:, (step_idx + 1) % 2:(step_idx + 1) % 2 + 1]
            # ig = i * (2*sg - 1) = 2*(i*sg) - i
            nc.vector.tensor_mul(st["ig"][:], gates[:, GI:GI + 1],
                                 gates[:, GG:GG + 1])
            nc.vector.scalar_tensor_tensor(st["ig"][:], st["ig"][:], 2.0,
                                           gates[:, GI:GI + 1],
                                           op0=mybir.AluOpType.mult,
                                           op1=mybir.AluOpType.subtract)
            nc.vector.tensor_tensor_scan(c_new, gates[:, GF:GF + 1], st["ig"][:],
                                         initial=c_prev,
                                         op0=mybir.AluOpType.mult,
                                         op1=mybir.AluOpType.add)
            # tanh(c) = 2*sigma(2c) - 1;  h = o*tanh(c) = 2*(o*s2c) - o
            nc.scalar.activation(st["tc"][:], c_new,
                                 mybir.ActivationFunctionType.Sigmoid, scale=2.0)
            nc.vector.tensor_mul(st["tc"][:], gates[:, GO:GO + 1], st["tc"][:])
            nc.vector.scalar_tensor_tensor(st["Ht"][:, t:t + 1], st["tc"][:], 2.0,
                                           gates[:, GO:GO + 1],
                                           op0=mybir.AluOpType.mult,
                                           op1=mybir.AluOpType.subtract)

        def run_layer(lf, lb, Gf, Gb):
            stf = make_dir_state(lf)
            stb = make_dir_state(lb)
            for i in range(T):
                dir_step(stf, Gf, i, i - 1 if i else None, i)
                dir_step(stb, Gb, T - 1 - i, T - i if i else None, i)
            H_out[lf] = stf["Ht"]
            H_out[lb] = stb["Ht"]

        run_layer("0f", "0b", G["0f"], G["0b"])

        if dbg:
            nc.sync.dma_start(dbg["G00f"].ap(), G["0f"][:])
            nc.sync.dma_start(dbg["H0"].ap()[:, 0:T], H_out["0f"][:])
            nc.sync.dma_start(dbg["H0"].ap()[:, T:2 * T], H_out["0b"][:])

        for d in ("f", "b"):
            rhs_tiles = [H_out["0f"][:], H_out["0b"][:]]
            lhs = [(lambda g, _h=half, _d=d:
                    S(f"wih1{_d}{_h}")[:, g * 64:(g + 1) * 64])
                   for half in ("F", "B")]
            G["1" + d] = input_proj("1" + d, rhs_tiles, lhs, tag=f"G1{d}")

        run_layer("1f", "1b", G["1f"], G["1b"])

        if dbg:
            nc.sync.dma_start(dbg["H1"].ap()[:, 0:T], H_out["1f"][:])
            nc.sync.dma_start(dbg["H1"].ap()[:, T:2 * T], H_out["1b"][:])

        # ---------------------------------------------------------- linear
        # feats_tc [T, 5] = H1f.T @ linF + H1b.T @ linB  (+ lin_b)
        fps = lpool.tile([T, NCLS], F32, tag="lps")
        nc.tensor.matmul(fps[:], H_out["1f"][:], S("linF"), start=True, stop=False)
        nc.tensor.matmul(fps[:], H_out["1b"][:], S("linB"), start=False, stop=True)
        feats = sb.tile([T, NCLS], F32, tag="feats")
        nc.vector.tensor_add(feats[:], fps[:], S("lin_b_rep"))

        if dbg:
            nc.sync.dma_start(dbg["feats"].ap(), feats[:])

        # ------------------------------------------------------------- CRF
        # Tree reduction in the log semiring, partition-parallel.  The CRF
        # scan is a chain of T log-matrix-products M_t
        # (M_t[i,j] = A[i,j] + lin_b[i] + rawfeat_t[i]); combine adjacent
        # pairs per level:  C = later (x) earlier,
        #   C[i,k] = LSE_j(later[i,j] + earlier[j,k]).
        # Layout: 4 time-consecutive matrices per partition row across 16
        # partitions; two in-row combine levels, one flatten DMA, then the
        # remaining levels on partition 0.
        def crf_ap(base, extra_off, dims):
            pdim = [list(base.ap[0])]
            return bass.AP(base.tensor, base.offset + extra_off,
                           pdim + [list(d) for d in dims])

        # feats_quad[m, 5q+i] = rawfeats[4m+q, i]   (psum, [16, 20])
        fq_ps = lpool.tile([16, 4 * NCLS], F32, tag="lps", name=f"fq_ps_{it}")
        h1f_q = H_out["1f"][:].rearrange("p (m q) -> p q m", q=4)
        h1b_q = H_out["1b"][:].rearrange("p (m q) -> p q m", q=4)
        for q in range(4):
            nc.tensor.matmul(fq_ps[:, 5 * q:5 * q + 5], h1f_q[:, q, :], S("linF"),
                             start=True, stop=False)
            nc.tensor.matmul(fq_ps[:, 5 * q:5 * q + 5], h1b_q[:, q, :], S("linB"),
                             start=False, stop=True)
        # af_quad[m, 25q + 5i + j] = (A[i,j] + lin_b[i]) + rawfeats[4m+q, i]
        af_quad = sb.tile([16, 100], F32, tag="af_quad")
        fq_b = crf_ap(fq_ps[:], 0, [[5, 4], [1, 5], [0, 5]])
        nc.vector.tensor_add(
            af_quad[:].rearrange("p (q i j) -> p q i j", i=NCLS, j=NCLS),
            S("A_lb_rep4", rows=16).rearrange("p (q i j) -> p q i j",
                                              i=NCLS, j=NCLS),
            fq_b)

        lv, mats, parts, lvl = af_quad, 4, 16, 0
        while mats > 1 or parts > 1:
            if mats == 1:
                flat = sb.tile([1, parts * 25], F32, tag="crf_flat")
                nc.sync.dma_start(flat[:], lv[:])
                lv, mats, parts = flat, parts, 1
            np_pairs = mats // 2
            base = lv[:]
            s_t = sb.tile([parts, np_pairs * 125], F32, tag=f"crf_s{lvl}",
                          name=f"crf_s{lvl}_{it}")
            # ISA limit: 3 free dims per DVE op -> one add per output row i
            for i in range(NCLS):
                out_s = crf_ap(s_t[:], 25 * i,
                               [[125, np_pairs], [5, 5], [1, 5]])
                later = crf_ap(base, 25 + 5 * i,
                               [[50, np_pairs], [0, 5], [1, 5]])
                earlier = crf_ap(base, 0,
                                 [[50, np_pairs], [1, 5], [5, 5]])
                nc.vector.tensor_add(out_s, later, earlier)
            mx_t = sb.tile([parts, np_pairs * 25], F32, tag=f"crf_m{lvl}",
                           name=f"crf_m{lvl}_{it}")
            nc.vector.tensor_reduce(
                mx_t[:], s_t[:].rearrange("o (r j) -> o r j", j=NCLS),
                axis=mybir.AxisListType.X, op=mybir.AluOpType.max)
            mx_b = crf_ap(mx_t[:], 0, [[1, np_pairs * 25], [0, 5]])
            nc.vector.tensor_sub(s_t[:].rearrange("o (r j) -> o r j", j=NCLS),
                                 s_t[:].rearrange("o (r j) -> o r j", j=NCLS),
                                 mx_b)
            nc.scalar.activation(s_t[:], s_t[:],
                                 mybir.ActivationFunctionType.Exp)
            se_t = sb.tile([parts, np_pairs * 25], F32, tag=f"crf_e{lvl}",
                           name=f"crf_e{lvl}_{it}")
            nc.vector.tensor_reduce(
                se_t[:], s_t[:].rearrange("o (r j) -> o r j", j=NCLS),
                axis=mybir.AxisListType.X, op=mybir.AluOpType.add)
            nc.scalar.activation(se_t[:], se_t[:],
                                 mybir.ActivationFunctionType.Ln)
            nxt = sb.tile([parts, np_pairs * 25], F32, tag=f"crf_n{lvl}",
                          name=f"crf_n{lvl}_{it}")
            nc.vector.tensor_add(nxt[:], mx_t[:], se_t[:])
            lv = nxt
            mats = np_pairs
            lvl += 1

        # alpha = M_tot (x) alpha0 :  alpha[i] = LSE_j(M_tot[i,j] + alpha0[j])
        alpha = sb.tile([1, NCLS], F32, tag="alpha")
        mx = sb.tile([1, NCLS], F32, tag="crf_m")
        se = sb.tile([1, NCLS], F32, tag="crf_se")
        av = sb.tile([1, 25], F32, tag="crf_av")
        a0b = crf_ap(S("alpha0", rows=1), 0, [[0, 5], [1, 5]])
        nc.vector.tensor_add(av[:].rearrange("o (i j) -> o i j", j=NCLS),
                             lv[:].rearrange("o (i j) -> o i j", j=NCLS), a0b)
        nc.vector.tensor_reduce(mx[:], av[:].rearrange("o (i j) -> o i j", j=NCLS),
                                axis=mybir.AxisListType.X, op=mybir.AluOpType.max)
        nc.vector.tensor_sub(av[:].rearrange("o (i j) -> o i j", j=NCLS),
                             av[:].rearrange("o (i j) -> o i j", j=NCLS),
                             crf_ap(mx[:], 0, [[1, 5], [0, 5]]))
        nc.scalar.activation(av[:], av[:], mybir.ActivationFunctionType.Exp)
        nc.vector.tensor_reduce(se[:], av[:].rearrange("o (i j) -> o i j", j=NCLS),
                                axis=mybir.AxisListType.X, op=mybir.AluOpType.add)
        nc.scalar.activation(se[:], se[:], mybir.ActivationFunctionType.Ln)
        nc.vector.tensor_add(alpha[:], mx[:], se[:])

        if dbg:
            nc.sync.dma_start(dbg["alpha"].ap(), alpha[:])

        # fwd = LSE(alpha + A[EOS, :])
        a_eos = S("A_rep", rows=1)[:, 5 * EOS:5 * EOS + 5]
        nc.vector.tensor_add(se[:], alpha[:], a_eos)   # reuse se as tmp [1,5]
        nc.vector.tensor_reduce(mx[:, 0:1], se[:], axis=mybir.AxisListType.X,
                                op=mybir.AluOpType.max)
        nm = sb.tile([1, 1], F32, tag="crf_nm")
        nc.scalar.mul(nm[:], mx[:, 0:1], -1.0)
        ex5 = sb.tile([1, NCLS], F32, tag="crf_ex5")
        sm1 = sb.tile([1, 1], F32, tag="crf_sm1")
        nc.scalar.activation(ex5[:], se[:], mybir.ActivationFunctionType.Exp,
                             bias=nm[:], accum_out=sm1[:])
        fwd = sb.tile([1, 1], F32, tag="fwd")
        nc.scalar.activation(fwd[:], sm1[:], mybir.ActivationFunctionType.Ln)
        nc.vector.tensor_add(fwd[:], fwd[:], mx[:, 0:1])

        # ------------------------------------------------------------- gold
        em = sb.tile([T, NCLS], F32, tag="em")
        nc.vector.tensor_mul(em[:], feats[:], S("onehot_em"))
        em_r = sb.tile([T, 1], F32, tag="em_r")
        nc.vector.tensor_reduce(em_r[:], em[:], axis=mybir.AxisListType.X,
                                op=mybir.AluOpType.add)
        gps = lpool.tile([1, 1], F32, tag="lps")
        nc.tensor.matmul(gps[:], em_r[:], S("ones64"), start=True, stop=True)

        tr = sb.tile([1, 25], F32, tag="tr")
        nc.vector.tensor_mul(tr[:], S("A_rep", rows=1), S("paircnt", rows=1))
        tr_s = sb.tile([1, 1], F32, tag="tr_s")
        nc.vector.tensor_reduce(tr_s[:], tr[:], axis=mybir.AxisListType.X,
                                op=mybir.AluOpType.add)

        # out = fwd - em_sum - tr_s + 0*chain
        res = sb.tile([1, 1], F32, tag="res")
        nc.vector.tensor_sub(res[:], fwd[:], gps[:])
        nc.vector.tensor_sub(res[:], res[:], tr_s[:])
        zc = sb.tile([1, 1], F32, tag="zc")
        nc.vector.tensor_scalar_mul(zc[:], chain_sb[:], 0.0)
        nc.vector.tensor_add(res[:], res[:], zc[:])
        return nc.sync.dma_start(out.ap(), res[:])


    prev = None
    for _it in range(iters):
        prev = one_iter(_it, prev)
    es.close()


# ======================================================================
# host side
# ======================================================================

def _prep_core_inputs(core, input_ids, tags, emb_t, wconv_t, lstm, lin_w,
                      lin_b, transition):
    """Build the per-core in_map (numpy only)."""
    ids_doc = np.ascontiguousarray(
        input_ids[core * T:(core + 1) * T].reshape(-1).astype(np.int32))
    ids_T = np.ascontiguousarray(ids_doc.reshape(NGT, 128).T)

    smalls = np.zeros((128, SMALL_COLS), np.float32)

    def put(name, arr, rows=None):
        lo, hi = _cols[name]
        r = arr.shape[0] if rows is None else rows
        smalls[0:r, lo:hi] = arr

    for li, (lname, d) in enumerate((("0", "f"), ("0", "b"),
                                     ("1", "f"), ("1", "b"))):
        Wih, Whh, bih, bhh = lstm[lname + d]
        put(f"whh{lname}{d}", _permute_gates(Whh.T))
        b = _permute_gates((bih + bhh))
        lo, hi = _cols["lb"]
        smalls[0:64, lo + 4 * li:lo + 4 * li + 4] = b.reshape(4, 64).T
        if lname == "1":
            WT = _permute_gates(Wih.T)  # (128, 256)
            put(f"wih1{d}F", WT[0:64])
            put(f"wih1{d}B", WT[64:128])

    linT = lin_w.T.astype(np.float32)  # (128, 5)
    put("linF", linT[0:64])
    put("linB", linT[64:128])
    put("lin_b_rep", np.tile(lin_b.astype(np.float32)[None, :], (T, 1)))

    tg = tags[:, core].astype(np.int64)  # (T,)
    onehot = np.zeros((T, NCLS), np.float32)
    onehot[np.arange(T), tg] = 1.0
    put("onehot_em", onehot)

    A = transition.astype(np.float32)
    put("A_rep", np.tile(A.reshape(1, 25), (T, 1)))
    alb = (A + lin_b.astype(np.float32)[:, None]).reshape(1, 25)
    put("A_lb_rep4", np.tile(alb, (16, 4)), rows=16)

    tg_ext = np.concatenate([[SOS], tg])
    paircnt = np.zeros((NCLS, NCLS), np.float32)
    for t in range(T):
        paircnt[tg_ext[t + 1], tg_ext[t]] += 1.0
    paircnt[EOS, tg_ext[T]] += 1.0  # fold A[EOS, last]
    put("paircnt", paircnt.reshape(1, 25), rows=1)

    a0 = np.full((1, NCLS), NEG, np.float32)
    a0[0, SOS] = 0.0
    put("alpha0", a0, rows=1)

    cb = np.stack([lstm["cb"][s] for s in SIZES])  # (3, 256)
    put("cb", cb.reshape(3, 2, 128).transpose(2, 0, 1).reshape(128, 6),
        rows=128)
    put("ones64", np.ones((64, 1), np.float32))

    m = {
        "emb": emb_t,
        "ids": ids_T,
        "smalls": smalls,
        "chain": np.zeros((1, 1), np.float32),
    }
    for s in SIZES:
        m[f"w{s}"] = wconv_t[s]
    for d in ("f", "b"):
        Wih = lstm["0" + d][0]
        m[f"wih0{d}"] = np.ascontiguousarray(_permute_gates(Wih.T))
    return m


_NC_CACHE = {}


def _get_nc(debug_outputs=False):
    key = (MODE, debug_outputs)
    if key not in _NC_CACHE:
        _NC_CACHE[key] = build_nc(debug_outputs)
    return _NC_CACHE[key]


def make_in_maps(inputs):
    """All 8 per-core input maps from the full problem inputs."""
    input_ids = np.asarray(inputs["input_ids"])
    tags = np.asarray(inputs["tags"])
    emb = np.asarray(inputs["emb"])
    transition = np.asarray(inputs["transition"])
    lin_w = np.asarray(inputs["lin_w"])
    lin_b = np.asarray(inputs["lin_b"])

    emb_t = np.ascontiguousarray(emb.astype(_emb_np_dtype()))
    wconv_t = {}
    for s in SIZES:
        w = np.asarray(inputs[f"cw{s}"])[:, 0]          # (256, s, 768)
        wconv_t[s] = np.ascontiguousarray(
            w.transpose(1, 2, 0).astype(_emb_np_dtype()))  # (s, 768, 256)

    lstm = {"cb": {s: np.asarray(inputs[f"cb{s}"]) for s in SIZES}}
    for lname in ("0", "1"):
        for d in ("f", "b"):
            lstm[lname + d] = tuple(
                np.asarray(inputs[f"{k}{lname}{d}"])
                for k in ("Wih", "Whh", "bih", "bhh"))

    return [
        _prep_core_inputs(c, input_ids, tags, emb_t, wconv_t, lstm,
                          lin_w, lin_b, transition)
        for c in range(B)
    ]


def kernel(**inputs):
    from concourse import bass_utils
    nc = _get_nc()
    in_maps = make_in_maps(inputs)
    res = bass_utils.run_bass_kernel_spmd(nc, in_maps, core_ids=list(range(B)))
    total = np.float32(0.0)
    for c in range(B):
        total += np.float32(res.results[c]["out"][0, 0])
    return np.asarray(total, dtype=np.float32)

